# revision 16
# baseline (speedup 1.0000x reference)
"""Trainium2 Bass kernel for nn_CustomGNN_66881230733874 (2-layer GAT + mean-pool + MLP).

Sharding: data-parallel over batch B=8 -> one graph per NeuronCore (8 cores).
Each core computes its full graph end-to-end (no collectives); host gathers [8,1].

V2.1 design notes:
  - ctx computed TRANSPOSED (ctx^T[u, n]) with V as the stationary matmul
    operand and E streaming as wide rhs: no per-step LDWEIGHTS reload of E
    chunks, no SBUF DMA-transposes of ctx.
  - whole datapath in fp16 (11-bit mantissa): single-matmul V and Wo paths
    (no split-hi/lo), Q/K/scores in fp16 as well.
  - layer1 softmax denominator: running sum of E tiles (DVE + gpsimd mix) +
    gpsimd partition_all_reduce, reciprocal_approx_fast (f32), normalize
    fused into the PSUM->SBUF drain.
  - layer2 denominator: ones-column in V_aug (65-wide lhsT), reciprocal of
    PSUM row 64, gpsimd partition_broadcast, fused normalize.
  - Wo2 + mean-pool folded after ctx2^T free-axis reduction.
  - prelude: A-mask pipeline and first head's scores start immediately;
    weight loads and projections are interleaved into the head loop.
"""

import numpy as np

import concourse.bass as bass
import concourse.mybir as mybir
import concourse.tile as tile
from concourse import bacc
from concourse import bass_isa
from concourse.bass_utils import run_bass_kernel_spmd
from concourse.masks import make_identity

F32 = mybir.dt.float32
BF16 = mybir.dt.bfloat16
FP16 = mybir.dt.float16
AF = mybir.ActivationFunctionType
OP = mybir.AluOpType

B = 8
N = 1024
F = 64
H = 8
U1, U2 = 128, 64
NT = N // 128  # 8 node chunks

WEIGHT_NAMES = [
    "Wq1", "Wk1", "Wv1", "Wo1", "Wq2", "Wk2", "Wv2", "Wo2",
    "W1", "b1", "W2", "b2", "W3", "b3",
]


def build_nc(repeats=1):
    nc = bacc.Bacc("TRN2", target_bir_lowering=False, debug=False)

    x_d = nc.dram_tensor("X", [N, F], F32, kind="ExternalInput")
    a_d = nc.dram_tensor("A", [N, N], F32, kind="ExternalInput")
    w_d = {}
    shapes = {
        "Wq1": [F, H * U1], "Wk1": [F, H * U1], "Wv1": [F, H * U1],
        "Wo1": [H * U1, U1],
        "Wq2": [U1, H * U2], "Wk2": [U1, H * U2], "Wv2": [U1, H * U2],
        "Wo2": [H * U2, U2],
        "W1": [F, 32], "b1": [32], "W2": [32, 16], "b2": [16],
        "W3": [16, 1], "b3": [1],
    }
    for k, s in shapes.items():
        w_d[k] = nc.dram_tensor(k, s, F32, kind="ExternalInput")
    y_d = nc.dram_tensor("y", [1, 1], F32, kind="ExternalOutput")

    with tile.TileContext(nc) as tc:
        with (
            tc.tile_pool(name="sb", bufs=1) as sb,
            tc.tile_pool(name="scratch", bufs=2) as scratch,
            tc.tile_pool(name="ps_s", bufs=2, space="PSUM") as ps_s,
            tc.tile_pool(name="ps_c", bufs=2, space="PSUM") as ps_c,
        ):
          for _rep in range(repeats):

            def load_w(name, dram, part, cols, kt=None, dt=FP16):
                """DMA f32 weight -> SBUF fp16 (optionally [128, kt, cols])."""
                if kt is None:
                    w32 = scratch.tile([part, cols], F32, tag="wscratch",
                                       bufs=2, name=f"w32_{name}")
                    nc.sync.dma_start(out=w32, in_=dram[:])
                    hi = sb.tile([part, cols], dt, tag=f"w_{name}", bufs=1,
                                 name=f"w_{name}")
                else:
                    w32 = scratch.tile([128, kt, cols], F32, tag="wscratch",
                                       bufs=2, name=f"w32_{name}")
                    nc.sync.dma_start(
                        out=w32, in_=dram[:].rearrange("(k p) c -> p k c", p=128))
                    hi = sb.tile([128, kt, cols], dt, tag=f"w_{name}", bufs=1,
                                 name=f"w_{name}")
                nc.vector.tensor_copy(out=hi, in_=w32)
                return hi

            # ---- A-mask pipeline first (longest prelude chain) ----
            mn = sb.tile([128, NT, N], FP16, tag="E", bufs=2)
            for cc in range(NT):
                a32c = scratch.tile([128, N], F32, tag="a32c", bufs=2)
                nc.sync.dma_start(
                    out=a32c,
                    in_=a_d[:].rearrange("(t p) n -> p t n", p=128)[:, cc, :])
                nc.vector.tensor_scalar(out=mn[:, cc, :], in0=a32c, scalar1=-1.0,
                                        scalar2=1.0, op0=OP.mult, op1=OP.add)
            Mt = sb.tile([128, NT, N], FP16, tag="Mt", bufs=1)
            for cc in range(NT):
                nc.sync.dma_start(out=Mt[:, :, cc * 128:(cc + 1) * 128],
                                  in_=mn[:, cc, :], transpose=True)

            # ---- X + XT [F, N] fp16 via PE transpose ----
            ident = sb.tile([128, 128], BF16, tag="ident", bufs=1)
            make_identity(nc, ident)
            x32 = scratch.tile([128, NT, F], F32, tag="a32c", bufs=2, name="x32")
            nc.sync.dma_start(out=x32, in_=x_d[:].rearrange("(t p) f -> p t f", p=128))
            xb = sb.tile([128, NT, F], BF16, tag="xb", bufs=1)
            nc.vector.tensor_copy(out=xb, in_=x32)
            XT = sb.tile([F, N], BF16, tag="XT", bufs=1)
            for t in range(NT):
                pt = ps_c.tile([F, 128], BF16, tag="ps_c", bufs=2)
                nc.tensor.transpose(pt, xb[:, t, :], ident)
                nc.scalar.copy(out=XT[:, t * 128:(t + 1) * 128], in_=pt)

            wq1 = load_w("Wq1", w_d["Wq1"], F, H * U1, dt=BF16)
            wk1 = load_w("Wk1", w_d["Wk1"], F, H * U1, dt=BF16)
            wv1 = load_w("Wv1", w_d["Wv1"], F, H * U1, dt=BF16)

            # ================= LAYER 1 (U=128) =================
            inv_s1 = 1.0 / float(np.sqrt(U1))

            QT1 = sb.tile([128, H, N], BF16, tag="QT1", bufs=1)
            KT1 = sb.tile([128, H, N], BF16, tag="KT1", bufs=1)
            qk_done = set()

            def qk_proj1(d):
                if d in qk_done or d >= H:
                    return
                qk_done.add(d)
                for w, dst, on_act in ((wq1, QT1, True), (wk1, KT1, False)):
                    for q in range(2):
                        p = ps_s.tile([128, 512], F32, tag="ps_s", bufs=2)
                        nc.tensor.matmul(p, lhsT=w[:, d * 128:(d + 1) * 128],
                                         rhs=XT[:, q * 512:(q + 1) * 512],
                                         start=True, stop=True)
                        dsl = dst[:, d, q * 512:(q + 1) * 512]
                        if on_act:
                            nc.scalar.copy(out=dsl, in_=p)
                        else:
                            nc.vector.tensor_copy(out=dsl, in_=p)

            # V1 [128, NT, H*U1] fp16 (m-part, (h,u)); c-chunk 0 = heads 0-3
            V1 = sb.tile([128, NT, H * U1], FP16, tag="V1", bufs=1)

            def v_proj1(c):
                for m in range(NT):
                    p = ps_c.tile([128, 512], F32, tag="ps_c", bufs=2,
                                  name="pv1")
                    nc.tensor.matmul(p, lhsT=XT[:, m * 128:(m + 1) * 128],
                                     rhs=wv1[:, c * 512:(c + 1) * 512],
                                     start=True, stop=True)
                    nc.scalar.copy(out=V1[:, m, c * 512:(c + 1) * 512], in_=p)

            qk_proj1(0)
            qk_proj1(1)
            v_proj1(0)

            CT1 = sb.tile([128, H, N], FP16, tag="CT1", bufs=1)
            E_t = {}
            z_t = {}

            def s_block1(h, m):
                """scores + exp + mask + z-partials for (head h, m-chunk)."""
                if m == 0:
                    E_t[h] = sb.tile([128, NT, N], FP16, tag="E", bufs=2,
                                     name=f"E{h}")
                E = E_t[h]
                s = ps_s.tile([128, 1024], F32, tag="ps_s", bufs=2)
                for q in range(2):
                    nc.tensor.matmul(
                        s[:, q * 512:(q + 1) * 512],
                        lhsT=KT1[:, h, m * 128:(m + 1) * 128],
                        rhs=QT1[:, h, q * 512:(q + 1) * 512],
                        start=True, stop=True)
                nc.scalar.activation(out=E[:, m, :], in_=s, func=AF.Exp,
                                     scale=inv_s1)
                nc.vector.tensor_mul(out=E[:, m, :], in0=E[:, m, :],
                                     in1=Mt[:, m, :])
                with nc.allow_low_precision(
                        reason="fp16 softmax-denominator partials; <=1024 "
                               "positive O(1) terms, ~1e-3 rel"):
                    if m == 1:
                        zacc = sb.tile([128, N], FP16, tag="zacc", bufs=2)
                        z_t[h] = zacc
                        nc.vector.tensor_add(out=zacc, in0=E[:, 0, :],
                                             in1=E[:, 1, :])
                    elif m > 1:
                        zacc = z_t[h]
                        nc.vector.tensor_add(out=zacc, in0=zacc,
                                             in1=E[:, m, :])
                    if m == NT - 1:
                        zat = sb.tile([128, NT, 128], FP16, tag="zat", bufs=2)
                        nc.sync.dma_start(out=zat, in_=z_t.pop(h),
                                          transpose=True)
                        zsum = sb.tile([128, 32], F32, tag="zsum", bufs=2)
                        nc.vector.reduce_sum(out=zsum[:, 0:NT].rearrange(
                                                 "p (t x) -> p t x", x=1),
                                             in_=zat,
                                             axis=mybir.AxisListType.X)
                        zit = sb.tile([128, 128], FP16, tag="zit", bufs=2)
                        nc.vector.memset(zit[:, NT:], 0.0)
                        nc.vector.reciprocal(out=zit[:, 0:NT],
                                             in_=zsum[:, 0:NT])
                        zrow = sb.tile([128, 128], FP16, tag="zrow", bufs=2)
                        nc.sync.dma_start(out=zrow, in_=zit, transpose=True)
                        zl = sb.tile([1, N], FP16, tag="zl", bufs=2)
                        nc.sync.dma_start(
                            out=zl.rearrange("x (t c) -> x t c", t=NT),
                            in_=zrow[0:NT, :])
                        zinv = sb.tile([128, N], FP16, tag="zinv", bufs=2)
                        nc.gpsimd.partition_broadcast(out_ap=zinv, in_ap=zl,
                                                      channels=128)
                        z_t[(h, 'inv')] = zinv

            def ctx_block1(h, m, cp):
                E = E_t[h]
                for q in range(2):
                    nc.tensor.matmul(
                        cp[:, q * 512:(q + 1) * 512],
                        lhsT=V1[:, m, h * U1:(h + 1) * U1],
                        rhs=E[:, m, q * 512:(q + 1) * 512],
                        start=(m == 0), stop=(m == NT - 1))
                if m == NT - 1:
                    E_t.pop(h)
                    zinv = z_t.pop((h, 'inv'))
                    for q in range(2):
                        nc.vector.tensor_mul(
                            out=CT1[:, h, q * 512:(q + 1) * 512],
                            in0=cp[:, q * 512:(q + 1) * 512],
                            in1=zinv[:, q * 512:(q + 1) * 512])

            # deferred loads, spread across the layer-1 head loop
            deferred = {}

            def deferred_loads(h):
                if h == 1:
                    v_proj1(1)
                elif h == 2:
                    deferred['wo1'] = load_w("Wo1", w_d["Wo1"], None, U1,
                                             kt=H * U1 // 128)
                elif h == 3:
                    deferred['wq2'] = load_w("Wq2", w_d["Wq2"], U1, H * U2)
                    deferred['wk2'] = load_w("Wk2", w_d["Wk2"], U1, H * U2)
                elif h == 4:
                    deferred['wv2'] = load_w("Wv2", w_d["Wv2"], U1, H * U2)
                    deferred['wo2'] = load_w("Wo2", w_d["Wo2"], None, U2,
                                             kt=H * U2 // 128)
                elif h == 5:
                    for nm, shp in (("W1", [F, 32]), ("W2", [32, 16]),
                                    ("W3", [16, 1])):
                        t = sb.tile(shp, F32, tag=nm, bufs=1, name=nm)
                        nc.sync.dma_start(out=t, in_=w_d[nm][:])
                        deferred[nm] = t
                    for nm, p in (("b1", 32), ("b2", 16), ("b3", 1)):
                        t = sb.tile([p, 1], F32, tag=nm, bufs=1, name=nm)
                        nc.sync.dma_start(
                            out=t, in_=w_d[nm][:].rearrange("(p x) -> p x", x=1))
                        deferred[nm] = t

            for m in range(NT):
                s_block1(0, m)
            for h in range(H):
                qk_proj1(h + 2)
                deferred_loads(h)
                cp = ps_c.tile([128, 1024], F32, tag="ps_c", bufs=2)
                for m in range(NT):
                    if h + 1 < H:
                        s_block1(h + 1, m)
                    ctx_block1(h, m, cp)

            wo1 = deferred['wo1']
            wq2, wk2 = deferred['wq2'], deferred['wk2']
            wv2, wo2 = deferred['wv2'], deferred['wo2']
            w1, w2, w3 = deferred['W1'], deferred['W2'], deferred['W3']
            b1, b2, b3 = deferred['b1'], deferred['b2'], deferred['b3']

            # Wo1: H1T [128, N] fp16 = Wo1^T @ CT1
            H1T = sb.tile([128, N], FP16, tag="H1T", bufs=1)
            for q in range(2):
                wp = ps_s.tile([128, 512], F32, tag="ps_s", bufs=2)
                for k in range(H):
                    nc.tensor.matmul(wp, lhsT=wo1[:, k, :],
                                     rhs=CT1[:, k, q * 512:(q + 1) * 512],
                                     start=(k == 0), stop=(k == H - 1))
                nc.scalar.copy(out=H1T[:, q * 512:(q + 1) * 512], in_=wp)

            # ================= LAYER 2 (U=64) =================
            inv_s2 = 1.0 / float(np.sqrt(U2))
            DT2 = H * U2 // 128  # 4

            QT2 = sb.tile([128, DT2, N], FP16, tag="QT2", bufs=1)
            KT2 = sb.tile([128, DT2, N], FP16, tag="KT2", bufs=1)
            for d in range(DT2):
                for w, dst, on_act in ((wq2, QT2, True), (wk2, KT2, False)):
                    for q in range(2):
                        p = ps_s.tile([128, 512], F32, tag="ps_s", bufs=2)
                        nc.tensor.matmul(p, lhsT=w[:, d * 128:(d + 1) * 128],
                                         rhs=H1T[:, q * 512:(q + 1) * 512],
                                         start=True, stop=True)
                        dsl = dst[:, d, q * 512:(q + 1) * 512]
                        if on_act:
                            nc.scalar.copy(out=dsl, in_=p)
                        else:
                            nc.vector.tensor_copy(out=dsl, in_=p)

            # V2A [128, NT, H, 65] fp16 with ones column
            VW2 = U2 + 1
            V2A = sb.tile([128, NT, H, VW2], FP16, tag="V2A", bufs=1)
            nc.vector.memset(V2A[:, :, :, U2:U2 + 1], 1.0)
            for m in range(NT):
                p = ps_s.tile([128, 512], F32, tag="ps_s", bufs=2)
                nc.tensor.matmul(p, lhsT=H1T[:, m * 128:(m + 1) * 128],
                                 rhs=wv2, start=True, stop=True)
                nc.scalar.copy(out=V2A[:, m, :, 0:U2],
                               in_=p.rearrange("p (h x) -> p h x", h=H))

            CT2 = sb.tile([128, DT2, N], FP16, tag="CT2", bufs=1)

            def s_block2(h, m):
                if m == 0:
                    E_t[h] = sb.tile([128, NT, N], FP16, tag="E", bufs=2,
                                     name=f"E{h}")
                E = E_t[h]
                r0 = (h % 2) * U2
                kt = h // 2
                s = ps_s.tile([128, 1024], F32, tag="ps_s", bufs=2)
                for q in range(2):
                    nc.tensor.matmul(
                        s[:, q * 512:(q + 1) * 512],
                        lhsT=KT2[r0:r0 + U2, kt, m * 128:(m + 1) * 128],
                        rhs=QT2[r0:r0 + U2, kt, q * 512:(q + 1) * 512],
                        start=True, stop=True)
                nc.scalar.activation(out=E[:, m, :], in_=s, func=AF.Exp,
                                     scale=inv_s2)
                nc.vector.tensor_mul(out=E[:, m, :], in0=E[:, m, :],
                                     in1=Mt[:, m, :])

            def ctx_block2(h, m, cp):
                E = E_t[h]
                for q in range(2):
                    nc.tensor.matmul(
                        cp[0:VW2, q * 512:(q + 1) * 512],
                        lhsT=V2A[:, m, h, :],
                        rhs=E[:, m, q * 512:(q + 1) * 512],
                        start=(m == 0), stop=(m == NT - 1))
                if m == NT - 1:
                    E_t.pop(h)
                    zinvh = sb.tile([1, N], FP16, tag="zinv2h", bufs=2)
                    with nc.allow_low_precision(
                            reason="fp16 reciprocal of O(512) denominator"):
                        nc.vector.reciprocal(out=zinvh, in_=cp[U2:U2 + 1, :])
                    zbc = sb.tile([U2, N], FP16, tag="zbc2", bufs=1)
                    nc.gpsimd.partition_broadcast(out_ap=zbc, in_ap=zinvh,
                                                  channels=U2)
                    r0 = (h % 2) * U2
                    kt = h // 2
                    for q in range(2):
                        nc.vector.tensor_mul(
                            out=CT2[r0:r0 + U2, kt, q * 512:(q + 1) * 512],
                            in0=cp[0:U2, q * 512:(q + 1) * 512],
                            in1=zbc[:, q * 512:(q + 1) * 512])

            for m in range(NT):
                s_block2(0, m)
            for h in range(H):
                cp = ps_c.tile([128, 1024], F32, tag="ps_c", bufs=2)
                for m in range(NT):
                    if h + 1 < H:
                        s_block2(h + 1, m)
                    ctx_block2(h, m, cp)

            # ---- mean pool (over n) then Wo2 fold + MLP ----
            pooled = sb.tile([128, DT2], F32, tag="pooled", bufs=1)
            for k in range(DT2):
                nc.vector.reduce_sum(out=pooled[:, k:k + 1], in_=CT2[:, k, :],
                                     axis=mybir.AxisListType.X)
            pooledh = sb.tile([128, DT2], FP16, tag="pooledh", bufs=1)
            nc.vector.tensor_copy(out=pooledh, in_=pooled)
            p0 = ps_c.tile([U2, 1], F32, tag="ps_c", bufs=2)
            for k in range(DT2):
                nc.tensor.matmul(p0, lhsT=wo2[:, k, :], rhs=pooledh[:, k:k + 1],
                                 start=(k == 0), stop=(k == DT2 - 1))
            hs = sb.tile([U2, 1], F32, tag="hs", bufs=1)
            nc.scalar.copy(out=hs, in_=p0)
            p1 = ps_c.tile([32, 1], F32, tag="ps_c", bufs=2)
            nc.tensor.matmul(p1, lhsT=w1, rhs=hs, start=True, stop=True)
            a1 = sb.tile([32, 1], F32, tag="a1", bufs=1)
            nc.scalar.activation(out=a1, in_=p1, func=AF.Relu, bias=b1,
                                 scale=1.0 / float(N))
            p2 = ps_c.tile([16, 1], F32, tag="ps_c", bufs=2)
            nc.tensor.matmul(p2, lhsT=w2, rhs=a1, start=True, stop=True)
            a2 = sb.tile([16, 1], F32, tag="a2", bufs=1)
            nc.scalar.activation(out=a2, in_=p2, func=AF.Relu, bias=b2)
            p3 = ps_c.tile([1, 1], F32, tag="ps_c", bufs=2)
            nc.tensor.matmul(p3, lhsT=w3, rhs=a2, start=True, stop=True)
            yt = sb.tile([1, 1], F32, tag="yt", bufs=1)
            nc.vector.tensor_add(out=yt, in0=p3, in1=b3)
            nc.sync.dma_start(out=y_d[:], in_=yt)

    nc.compile()
    return nc


_NC = None


def _get_nc():
    global _NC
    if _NC is None:
        _NC = build_nc()
    return _NC


def make_in_maps(inputs):
    in_maps = []
    for i in range(B):
        m = {"X": np.ascontiguousarray(np.asarray(inputs["X"][i], dtype=np.float32)),
             "A": np.ascontiguousarray(np.asarray(inputs["A"][i], dtype=np.float32))}
        for k in WEIGHT_NAMES:
            m[k] = np.ascontiguousarray(np.asarray(inputs[k], dtype=np.float32))
        in_maps.append(m)
    return in_maps


def run(inputs, trace=False):
    nc = _get_nc()
    res = run_bass_kernel_spmd(nc, make_in_maps(inputs), list(range(B)), trace=trace)
    y = np.stack([res.results[i]["y"][0] for i in range(B)], axis=0)
    return y.astype(np.float32), res


def kernel(**inputs):
    y, _ = run(inputs, trace=False)
    return y


# revision 17
# speedup vs baseline: 1.0967x; 1.0967x over previous
"""Trainium2 Bass kernel for nn_CustomGNN_66881230733874 (2-layer GAT + mean-pool + MLP).

Sharding: data-parallel over batch B=8 -> one graph per NeuronCore (8 cores).
Each core computes its full graph end-to-end (no collectives); host gathers [8,1].

V2.1 design notes:
  - ctx computed TRANSPOSED (ctx^T[u, n]) with V as the stationary matmul
    operand and E streaming as wide rhs: no per-step LDWEIGHTS reload of E
    chunks, no SBUF DMA-transposes of ctx.
  - whole datapath in fp16 (11-bit mantissa): single-matmul V and Wo paths
    (no split-hi/lo), Q/K/scores in fp16 as well.
  - layer1 softmax denominator: running sum of E tiles (DVE + gpsimd mix) +
    gpsimd partition_all_reduce, reciprocal_approx_fast (f32), normalize
    fused into the PSUM->SBUF drain.
  - layer2 denominator: ones-column in V_aug (65-wide lhsT), reciprocal of
    PSUM row 64, gpsimd partition_broadcast, fused normalize.
  - Wo2 + mean-pool folded after ctx2^T free-axis reduction.
  - prelude: A-mask pipeline and first head's scores start immediately;
    weight loads and projections are interleaved into the head loop.
"""

import numpy as np

import concourse.bass as bass
import concourse.mybir as mybir
import concourse.tile as tile
from concourse import bacc
from concourse import bass_isa
from concourse.bass_utils import run_bass_kernel_spmd
from concourse.masks import make_identity

F32 = mybir.dt.float32
BF16 = mybir.dt.bfloat16
FP16 = mybir.dt.float16
AF = mybir.ActivationFunctionType
OP = mybir.AluOpType

B = 8
N = 1024
F = 64
H = 8
U1, U2 = 128, 64
NT = N // 128  # 8 node chunks

WEIGHT_NAMES = [
    "Wq1", "Wk1", "Wv1", "Wo1", "Wq2", "Wk2", "Wv2", "Wo2",
    "W1", "b1", "W2", "b2", "W3", "b3",
]


def build_nc(repeats=1):
    nc = bacc.Bacc("TRN2", target_bir_lowering=False, debug=False)

    x_d = nc.dram_tensor("X", [N, F], F32, kind="ExternalInput")
    a_d = nc.dram_tensor("A", [N, N], F32, kind="ExternalInput")
    w_d = {}
    shapes = {
        "Wq1": [F, H * U1], "Wk1": [F, H * U1], "Wv1": [F, H * U1],
        "Wo1": [H * U1, U1],
        "Wq2": [U1, H * U2], "Wk2": [U1, H * U2], "Wv2": [U1, H * U2],
        "Wo2": [H * U2, U2],
        "W1": [F, 32], "b1": [32], "W2": [32, 16], "b2": [16],
        "W3": [16, 1], "b3": [1],
    }
    for k, s in shapes.items():
        w_d[k] = nc.dram_tensor(k, s, F32, kind="ExternalInput")
    y_d = nc.dram_tensor("y", [1, 1], F32, kind="ExternalOutput")

    with tile.TileContext(nc) as tc:
        with (
            tc.tile_pool(name="sb", bufs=1) as sb,
            tc.tile_pool(name="scratch", bufs=2) as scratch,
            tc.tile_pool(name="ps_s", bufs=2, space="PSUM") as ps_s,
            tc.tile_pool(name="ps_c", bufs=2, space="PSUM") as ps_c,
        ):
          for _rep in range(repeats):

            def load_w(name, dram, part, cols, kt=None, dt=FP16):
                """DMA f32 weight -> SBUF fp16 (optionally [128, kt, cols])."""
                if kt is None:
                    w32 = scratch.tile([part, cols], F32, tag="wscratch",
                                       bufs=2, name=f"w32_{name}")
                    nc.sync.dma_start(out=w32, in_=dram[:])
                    hi = sb.tile([part, cols], dt, tag=f"w_{name}", bufs=1,
                                 name=f"w_{name}")
                else:
                    w32 = scratch.tile([128, kt, cols], F32, tag="wscratch",
                                       bufs=2, name=f"w32_{name}")
                    nc.sync.dma_start(
                        out=w32, in_=dram[:].rearrange("(k p) c -> p k c", p=128))
                    hi = sb.tile([128, kt, cols], dt, tag=f"w_{name}", bufs=1,
                                 name=f"w_{name}")
                nc.vector.tensor_copy(out=hi, in_=w32)
                return hi

            ident = sb.tile([128, 128], BF16, tag="ident", bufs=1)
            make_identity(nc, ident)
            x32 = scratch.tile([128, NT, F], F32, tag="x32", bufs=1, name="x32")
            nc.sync.dma_start(out=x32, in_=x_d[:].rearrange("(t p) f -> p t f", p=128))

            # ---- A-mask pipeline (longest prelude chain) ----
            mn = sb.tile([128, NT, N], FP16, tag="E", bufs=2)
            for cc in range(NT):
                a32c = scratch.tile([128, N], F32, tag="a32c", bufs=2)
                nc.sync.dma_start(
                    out=a32c,
                    in_=a_d[:].rearrange("(t p) n -> p t n", p=128)[:, cc, :])
                nc.vector.tensor_scalar(out=mn[:, cc, :], in0=a32c, scalar1=-1.0,
                                        scalar2=1.0, op0=OP.mult, op1=OP.add)
            Mt = sb.tile([128, NT, N], FP16, tag="Mt", bufs=1)
            for cc in range(NT):
                nc.sync.dma_start(out=Mt[:, :, cc * 128:(cc + 1) * 128],
                                  in_=mn[:, cc, :], transpose=True)

            # ---- XT [F, N] bf16 via PE transpose ----

            xb = sb.tile([128, NT, F], BF16, tag="xb", bufs=1)
            nc.vector.tensor_copy(out=xb, in_=x32)
            XT = sb.tile([F, N], BF16, tag="XT", bufs=1)
            for t in range(NT):
                pt = ps_c.tile([F, 128], BF16, tag="ps_c", bufs=2)
                nc.tensor.transpose(pt, xb[:, t, :], ident)
                nc.scalar.copy(out=XT[:, t * 128:(t + 1) * 128], in_=pt)

            wq1 = load_w("Wq1", w_d["Wq1"], F, H * U1, dt=BF16)
            wk1 = load_w("Wk1", w_d["Wk1"], F, H * U1, dt=BF16)
            wv1 = load_w("Wv1", w_d["Wv1"], F, H * U1, dt=BF16)

            # ================= LAYER 1 (U=128) =================
            inv_s1 = 1.0 / float(np.sqrt(U1))

            QT1 = sb.tile([128, H, N], BF16, tag="QT1", bufs=1)
            KT1 = sb.tile([128, H, N], BF16, tag="KT1", bufs=1)
            qk_done = set()

            def qk_proj1(d):
                if d in qk_done or d >= H:
                    return
                qk_done.add(d)
                for w, dst, on_act in ((wq1, QT1, True), (wk1, KT1, False)):
                    for q in range(2):
                        p = ps_s.tile([128, 512], F32, tag="ps_s", bufs=2)
                        nc.tensor.matmul(p, lhsT=w[:, d * 128:(d + 1) * 128],
                                         rhs=XT[:, q * 512:(q + 1) * 512],
                                         start=True, stop=True)
                        dsl = dst[:, d, q * 512:(q + 1) * 512]
                        if on_act:
                            nc.scalar.copy(out=dsl, in_=p)
                        else:
                            nc.vector.tensor_copy(out=dsl, in_=p)

            # V1 [128, NT, H*U1] fp16 (m-part, (h,u)); c-chunk 0 = heads 0-3
            V1 = sb.tile([128, NT, H * U1], FP16, tag="V1", bufs=1)

            def v_proj1(c):
                for m in range(NT):
                    p = ps_c.tile([128, 512], F32, tag="ps_c", bufs=2,
                                  name="pv1")
                    nc.tensor.matmul(p, lhsT=XT[:, m * 128:(m + 1) * 128],
                                     rhs=wv1[:, c * 512:(c + 1) * 512],
                                     start=True, stop=True)
                    nc.scalar.copy(out=V1[:, m, c * 512:(c + 1) * 512], in_=p)

            qk_proj1(0)
            qk_proj1(1)
            v_proj1(0)

            CT1 = sb.tile([128, H, N], FP16, tag="CT1", bufs=1)
            E_t = {}
            z_t = {}

            def s_block1(h, m):
                """scores + exp + mask + z-partials for (head h, m-chunk)."""
                if m == 0:
                    E_t[h] = sb.tile([128, NT, N], FP16, tag="E", bufs=2,
                                     name=f"E{h}")
                E = E_t[h]
                s = ps_s.tile([128, 1024], F32, tag="ps_s", bufs=2)
                for q in range(2):
                    nc.tensor.matmul(
                        s[:, q * 512:(q + 1) * 512],
                        lhsT=KT1[:, h, m * 128:(m + 1) * 128],
                        rhs=QT1[:, h, q * 512:(q + 1) * 512],
                        start=True, stop=True)
                nc.scalar.activation(out=E[:, m, :], in_=s, func=AF.Exp,
                                     scale=inv_s1)
                nc.vector.tensor_mul(out=E[:, m, :], in0=E[:, m, :],
                                     in1=Mt[:, m, :])
                with nc.allow_low_precision(
                        reason="fp16 softmax-denominator partials; <=1024 "
                               "positive O(1) terms, ~1e-3 rel"):
                    if m == 1:
                        zacc = sb.tile([128, N], FP16, tag="zacc", bufs=2)
                        z_t[h] = zacc
                        nc.vector.tensor_add(out=zacc, in0=E[:, 0, :],
                                             in1=E[:, 1, :])
                    elif m > 1:
                        zacc = z_t[h]
                        nc.vector.tensor_add(out=zacc, in0=zacc,
                                             in1=E[:, m, :])
                    if m == NT - 1:
                        zat = sb.tile([128, NT, 128], FP16, tag="zat", bufs=2)
                        nc.sync.dma_start(out=zat, in_=z_t.pop(h),
                                          transpose=True)
                        zsum = sb.tile([128, 32], F32, tag="zsum", bufs=2)
                        nc.vector.reduce_sum(out=zsum[:, 0:NT].rearrange(
                                                 "p (t x) -> p t x", x=1),
                                             in_=zat,
                                             axis=mybir.AxisListType.X)
                        zit = sb.tile([128, 128], FP16, tag="zit", bufs=2)
                        nc.vector.memset(zit[:, NT:], 0.0)
                        nc.vector.reciprocal(out=zit[:, 0:NT],
                                             in_=zsum[:, 0:NT])
                        zrow = sb.tile([128, 128], FP16, tag="zrow", bufs=2)
                        nc.sync.dma_start(out=zrow, in_=zit, transpose=True)
                        zl = sb.tile([1, N], FP16, tag="zl", bufs=2)
                        nc.sync.dma_start(
                            out=zl.rearrange("x (t c) -> x t c", t=NT),
                            in_=zrow[0:NT, :])
                        zinv = sb.tile([128, N], FP16, tag="zinv", bufs=2)
                        nc.gpsimd.partition_broadcast(out_ap=zinv, in_ap=zl,
                                                      channels=128)
                        z_t[(h, 'inv')] = zinv

            def ctx_block1(h, m, cp):
                E = E_t[h]
                for q in range(2):
                    nc.tensor.matmul(
                        cp[:, q * 512:(q + 1) * 512],
                        lhsT=V1[:, m, h * U1:(h + 1) * U1],
                        rhs=E[:, m, q * 512:(q + 1) * 512],
                        start=(m == 0), stop=(m == NT - 1))
                if m == NT - 1:
                    E_t.pop(h)
                    zinv = z_t.pop((h, 'inv'))
                    for q in range(2):
                        nc.vector.tensor_mul(
                            out=CT1[:, h, q * 512:(q + 1) * 512],
                            in0=cp[:, q * 512:(q + 1) * 512],
                            in1=zinv[:, q * 512:(q + 1) * 512])

            # deferred loads, spread across the layer-1 head loop
            deferred = {}

            def deferred_loads(h):
                if h == 1:
                    v_proj1(1)
                elif h == 2:
                    deferred['wo1'] = load_w("Wo1", w_d["Wo1"], None, U1,
                                             kt=H * U1 // 128)
                elif h == 3:
                    deferred['wq2'] = load_w("Wq2", w_d["Wq2"], U1, H * U2)
                    deferred['wk2'] = load_w("Wk2", w_d["Wk2"], U1, H * U2)
                elif h == 4:
                    deferred['wv2'] = load_w("Wv2", w_d["Wv2"], U1, H * U2)
                    deferred['wo2'] = load_w("Wo2", w_d["Wo2"], None, U2,
                                             kt=H * U2 // 128)
                elif h == 5:
                    for nm, shp in (("W1", [F, 32]), ("W2", [32, 16]),
                                    ("W3", [16, 1])):
                        t = sb.tile(shp, F32, tag=nm, bufs=1, name=nm)
                        nc.sync.dma_start(out=t, in_=w_d[nm][:])
                        deferred[nm] = t
                    for nm, p in (("b1", 32), ("b2", 16), ("b3", 1)):
                        t = sb.tile([p, 1], F32, tag=nm, bufs=1, name=nm)
                        nc.sync.dma_start(
                            out=t, in_=w_d[nm][:].rearrange("(p x) -> p x", x=1))
                        deferred[nm] = t

            for m in range(NT):
                s_block1(0, m)
            for h in range(H):
                qk_proj1(h + 2)
                deferred_loads(h)
                cp = ps_c.tile([128, 1024], F32, tag="ps_c", bufs=2)
                for m in range(NT):
                    if h + 1 < H:
                        s_block1(h + 1, m)
                    ctx_block1(h, m, cp)

            wo1 = deferred['wo1']
            wq2, wk2 = deferred['wq2'], deferred['wk2']
            wv2, wo2 = deferred['wv2'], deferred['wo2']
            w1, w2, w3 = deferred['W1'], deferred['W2'], deferred['W3']
            b1, b2, b3 = deferred['b1'], deferred['b2'], deferred['b3']

            # Wo1: H1T [128, N] fp16 = Wo1^T @ CT1
            H1T = sb.tile([128, N], FP16, tag="H1T", bufs=1)
            for q in range(2):
                wp = ps_s.tile([128, 512], F32, tag="ps_s", bufs=2)
                for k in range(H):
                    nc.tensor.matmul(wp, lhsT=wo1[:, k, :],
                                     rhs=CT1[:, k, q * 512:(q + 1) * 512],
                                     start=(k == 0), stop=(k == H - 1))
                nc.scalar.copy(out=H1T[:, q * 512:(q + 1) * 512], in_=wp)

            # ================= LAYER 2 (U=64) =================
            inv_s2 = 1.0 / float(np.sqrt(U2))
            DT2 = H * U2 // 128  # 4

            QT2 = sb.tile([128, DT2, N], FP16, tag="QT2", bufs=1)
            KT2 = sb.tile([128, DT2, N], FP16, tag="KT2", bufs=1)
            for d in range(DT2):
                for w, dst in ((wq2, QT2), (wk2, KT2)):
                    for q in range(2):
                        p = ps_s.tile([128, 512], F32, tag="ps_s", bufs=2)
                        nc.tensor.matmul(p, lhsT=w[:, d * 128:(d + 1) * 128],
                                         rhs=H1T[:, q * 512:(q + 1) * 512],
                                         start=True, stop=True)
                        nc.scalar.copy(out=dst[:, d, q * 512:(q + 1) * 512],
                                       in_=p)

            # V2 [128, NT, H*U2] fp16
            V2 = sb.tile([128, NT, H * U2], FP16, tag="V2A", bufs=1)
            for m in range(NT):
                p = ps_s.tile([128, 512], F32, tag="ps_s", bufs=2)
                nc.tensor.matmul(p, lhsT=H1T[:, m * 128:(m + 1) * 128],
                                 rhs=wv2, start=True, stop=True)
                nc.scalar.copy(out=V2[:, m, :], in_=p)

            CT2 = sb.tile([128, DT2, N], FP16, tag="CT2", bufs=1)

            def s_block2(h, m):
                if m == 0:
                    E_t[h] = sb.tile([128, NT, N], FP16, tag="E", bufs=2,
                                     name=f"E{h}")
                E = E_t[h]
                r0 = (h % 2) * U2
                kt = h // 2
                s = ps_s.tile([128, 1024], F32, tag="ps_s", bufs=2)
                for q in range(2):
                    nc.tensor.matmul(
                        s[:, q * 512:(q + 1) * 512],
                        lhsT=KT2[r0:r0 + U2, kt, m * 128:(m + 1) * 128],
                        rhs=QT2[r0:r0 + U2, kt, q * 512:(q + 1) * 512],
                        start=True, stop=True)
                nc.scalar.activation(out=E[:, m, :], in_=s, func=AF.Exp,
                                     scale=inv_s2)
                nc.vector.tensor_mul(out=E[:, m, :], in0=E[:, m, :],
                                     in1=Mt[:, m, :])
                with nc.allow_low_precision(
                        reason="fp16 softmax-denominator partials"):
                    if m == 1:
                        zacc = sb.tile([128, N], FP16, tag="zacc", bufs=2)
                        z_t[h] = zacc
                        nc.vector.tensor_add(out=zacc, in0=E[:, 0, :],
                                             in1=E[:, 1, :])
                    elif m > 1:
                        nc.vector.tensor_add(out=z_t[h], in0=z_t[h],
                                             in1=E[:, m, :])
                    if m == NT - 1:
                        zat = sb.tile([128, NT, 128], FP16, tag="zat", bufs=2)
                        nc.sync.dma_start(out=zat, in_=z_t.pop(h),
                                          transpose=True)
                        zsum = sb.tile([128, 32], F32, tag="zsum", bufs=2)
                        nc.vector.reduce_sum(out=zsum[:, 0:NT].rearrange(
                                                 "p (t x) -> p t x", x=1),
                                             in_=zat,
                                             axis=mybir.AxisListType.X)
                        zit = sb.tile([128, 128], FP16, tag="zit", bufs=2)
                        nc.vector.memset(zit[:, NT:], 0.0)
                        nc.vector.reciprocal(out=zit[:, 0:NT],
                                             in_=zsum[:, 0:NT])
                        zrow = sb.tile([128, 128], FP16, tag="zrow", bufs=2)
                        nc.sync.dma_start(out=zrow, in_=zit, transpose=True)
                        zl = sb.tile([1, N], FP16, tag="zl", bufs=2)
                        nc.sync.dma_start(
                            out=zl.rearrange("x (t c) -> x t c", t=NT),
                            in_=zrow[0:NT, :])
                        zinv = sb.tile([128, N], FP16, tag="zinv", bufs=2)
                        nc.gpsimd.partition_broadcast(out_ap=zinv, in_ap=zl,
                                                      channels=128)
                        z_t[(h, 'inv')] = zinv

            def ctx_block2(h, m, cp):
                E = E_t[h]
                for q in range(2):
                    nc.tensor.matmul(
                        cp[0:U2, q * 512:(q + 1) * 512],
                        lhsT=V2[:, m, h * U2:(h + 1) * U2],
                        rhs=E[:, m, q * 512:(q + 1) * 512],
                        start=(m == 0), stop=(m == NT - 1))
                if m == NT - 1:
                    E_t.pop(h)
                    zinv = z_t.pop((h, 'inv'))
                    r0 = (h % 2) * U2
                    kt = h // 2
                    for q in range(2):
                        nc.vector.tensor_mul(
                            out=CT2[r0:r0 + U2, kt, q * 512:(q + 1) * 512],
                            in0=cp[0:U2, q * 512:(q + 1) * 512],
                            in1=zinv[0:U2, q * 512:(q + 1) * 512])

            for m in range(NT):
                s_block2(0, m)
            for h in range(H):
                cp = ps_c.tile([128, 1024], F32, tag="ps_c", bufs=2)
                for m in range(NT):
                    if h + 1 < H:
                        s_block2(h + 1, m)
                    ctx_block2(h, m, cp)

            # ---- mean pool (over n) then Wo2 fold + MLP ----
            pooled = sb.tile([128, DT2], F32, tag="pooled", bufs=1)
            for k in range(DT2):
                nc.vector.reduce_sum(out=pooled[:, k:k + 1], in_=CT2[:, k, :],
                                     axis=mybir.AxisListType.X)
            pooledh = sb.tile([128, DT2], FP16, tag="pooledh", bufs=1)
            nc.vector.tensor_copy(out=pooledh, in_=pooled)
            p0 = ps_c.tile([U2, 1], F32, tag="ps_c", bufs=2)
            for k in range(DT2):
                nc.tensor.matmul(p0, lhsT=wo2[:, k, :], rhs=pooledh[:, k:k + 1],
                                 start=(k == 0), stop=(k == DT2 - 1))
            hs = sb.tile([U2, 1], F32, tag="hs", bufs=1)
            nc.scalar.copy(out=hs, in_=p0)
            p1 = ps_c.tile([32, 1], F32, tag="ps_c", bufs=2)
            nc.tensor.matmul(p1, lhsT=w1, rhs=hs, start=True, stop=True)
            a1 = sb.tile([32, 1], F32, tag="a1", bufs=1)
            nc.scalar.activation(out=a1, in_=p1, func=AF.Relu, bias=b1,
                                 scale=1.0 / float(N))
            p2 = ps_c.tile([16, 1], F32, tag="ps_c", bufs=2)
            nc.tensor.matmul(p2, lhsT=w2, rhs=a1, start=True, stop=True)
            a2 = sb.tile([16, 1], F32, tag="a2", bufs=1)
            nc.scalar.activation(out=a2, in_=p2, func=AF.Relu, bias=b2)
            p3 = ps_c.tile([1, 1], F32, tag="ps_c", bufs=2)
            nc.tensor.matmul(p3, lhsT=w3, rhs=a2, start=True, stop=True)
            yt = sb.tile([1, 1], F32, tag="yt", bufs=1)
            nc.vector.tensor_add(out=yt, in0=p3, in1=b3)
            nc.sync.dma_start(out=y_d[:], in_=yt)

    nc.compile()
    return nc


_NC = None


def _get_nc():
    global _NC
    if _NC is None:
        _NC = build_nc()
    return _NC


def make_in_maps(inputs):
    in_maps = []
    for i in range(B):
        m = {"X": np.ascontiguousarray(np.asarray(inputs["X"][i], dtype=np.float32)),
             "A": np.ascontiguousarray(np.asarray(inputs["A"][i], dtype=np.float32))}
        for k in WEIGHT_NAMES:
            m[k] = np.ascontiguousarray(np.asarray(inputs[k], dtype=np.float32))
        in_maps.append(m)
    return in_maps


def run(inputs, trace=False):
    nc = _get_nc()
    res = run_bass_kernel_spmd(nc, make_in_maps(inputs), list(range(B)), trace=trace)
    y = np.stack([res.results[i]["y"][0] for i in range(B)], axis=0)
    return y.astype(np.float32), res


def kernel(**inputs):
    y, _ = run(inputs, trace=False)
    return y


# revision 18
# speedup vs baseline: 1.1138x; 1.0156x over previous
"""Trainium2 Bass kernel for nn_CustomGNN_66881230733874 (2-layer GAT + mean-pool + MLP).

Sharding: data-parallel over batch B=8 -> one graph per NeuronCore (8 cores).
Each core computes its full graph end-to-end (no collectives); host gathers [8,1].

V2.1 design notes:
  - ctx computed TRANSPOSED (ctx^T[u, n]) with V as the stationary matmul
    operand and E streaming as wide rhs: no per-step LDWEIGHTS reload of E
    chunks, no SBUF DMA-transposes of ctx.
  - whole datapath in fp16 (11-bit mantissa): single-matmul V and Wo paths
    (no split-hi/lo), Q/K/scores in fp16 as well.
  - layer1 softmax denominator: running sum of E tiles (DVE + gpsimd mix) +
    gpsimd partition_all_reduce, reciprocal_approx_fast (f32), normalize
    fused into the PSUM->SBUF drain.
  - layer2 denominator: ones-column in V_aug (65-wide lhsT), reciprocal of
    PSUM row 64, gpsimd partition_broadcast, fused normalize.
  - Wo2 + mean-pool folded after ctx2^T free-axis reduction.
  - prelude: A-mask pipeline and first head's scores start immediately;
    weight loads and projections are interleaved into the head loop.
"""

import numpy as np

import concourse.bass as bass
import concourse.mybir as mybir
import concourse.tile as tile
from concourse import bacc
from concourse import bass_isa
from concourse.bass_utils import run_bass_kernel_spmd
from concourse.masks import make_identity

F32 = mybir.dt.float32
BF16 = mybir.dt.bfloat16
FP16 = mybir.dt.float16
AF = mybir.ActivationFunctionType
OP = mybir.AluOpType

B = 8
N = 1024
F = 64
H = 8
U1, U2 = 128, 64
NT = N // 128  # 8 node chunks

WEIGHT_NAMES = [
    "Wq1", "Wk1", "Wv1", "Wo1", "Wq2", "Wk2", "Wv2", "Wo2",
    "W1", "b1", "W2", "b2", "W3", "b3",
]


def build_nc(repeats=1):
    nc = bacc.Bacc("TRN2", target_bir_lowering=False, debug=False)

    x_d = nc.dram_tensor("X", [N, F], F32, kind="ExternalInput")
    a_d = nc.dram_tensor("A", [N, N], F32, kind="ExternalInput")
    w_d = {}
    shapes = {
        "Wq1": [F, H * U1], "Wk1": [F, H * U1], "Wv1": [F, H * U1],
        "Wo1": [H * U1, U1],
        "Wq2": [U1, H * U2], "Wk2": [U1, H * U2], "Wv2": [U1, H * U2],
        "Wo2": [H * U2, U2],
        "W1": [F, 32], "b1": [32], "W2": [32, 16], "b2": [16],
        "W3": [16, 1], "b3": [1],
    }
    for k, s in shapes.items():
        w_d[k] = nc.dram_tensor(k, s, F32, kind="ExternalInput")
    y_d = nc.dram_tensor("y", [1, 1], F32, kind="ExternalOutput")

    with tile.TileContext(nc) as tc:
        with (
            tc.tile_pool(name="sb", bufs=1) as sb,
            tc.tile_pool(name="scratch", bufs=2) as scratch,
            tc.tile_pool(name="ps_s", bufs=2, space="PSUM") as ps_s,
            tc.tile_pool(name="ps_c", bufs=2, space="PSUM") as ps_c,
        ):
          for _rep in range(repeats):

            def load_w(name, dram, part, cols, kt=None, dt=FP16):
                """DMA f32 weight -> SBUF fp16 (optionally [128, kt, cols])."""
                if kt is None:
                    w32 = scratch.tile([part, cols], F32, tag="wscratch",
                                       bufs=2, name=f"w32_{name}")
                    nc.sync.dma_start(out=w32, in_=dram[:])
                    hi = sb.tile([part, cols], dt, tag=f"w_{name}", bufs=1,
                                 name=f"w_{name}")
                else:
                    w32 = scratch.tile([128, kt, cols], F32, tag="wscratch",
                                       bufs=2, name=f"w32_{name}")
                    nc.sync.dma_start(
                        out=w32, in_=dram[:].rearrange("(k p) c -> p k c", p=128))
                    hi = sb.tile([128, kt, cols], dt, tag=f"w_{name}", bufs=1,
                                 name=f"w_{name}")
                nc.vector.tensor_copy(out=hi, in_=w32)
                return hi

            ident = sb.tile([128, 128], BF16, tag="ident", bufs=1)
            make_identity(nc, ident)
            x32 = scratch.tile([128, NT, F], F32, tag="x32", bufs=1, name="x32")
            nc.sync.dma_start(out=x32, in_=x_d[:].rearrange("(t p) f -> p t f", p=128))
            wq1 = load_w("Wq1", w_d["Wq1"], F, H * U1, dt=BF16)
            wk1 = load_w("Wk1", w_d["Wk1"], F, H * U1, dt=BF16)
            wv1 = load_w("Wv1", w_d["Wv1"], F, H * U1, dt=BF16)

            # ---- A-mask pipeline (longest prelude chain) ----
            mn = sb.tile([128, NT, N], FP16, tag="E", bufs=2)
            for cc in range(NT):
                a32c = scratch.tile([128, N], F32, tag="a32c", bufs=2)
                nc.sync.dma_start(
                    out=a32c,
                    in_=a_d[:].rearrange("(t p) n -> p t n", p=128)[:, cc, :])
                nc.vector.tensor_scalar(out=mn[:, cc, :], in0=a32c, scalar1=-1.0,
                                        scalar2=1.0, op0=OP.mult, op1=OP.add)
            Mt = sb.tile([128, NT, N], FP16, tag="Mt", bufs=1)
            for cc in range(NT):
                nc.sync.dma_start(out=Mt[:, :, cc * 128:(cc + 1) * 128],
                                  in_=mn[:, cc, :], transpose=True)

            # ---- XT [F, N] bf16 via PE transpose ----

            xb = sb.tile([128, NT, F], BF16, tag="xb", bufs=1)
            nc.vector.tensor_copy(out=xb, in_=x32)
            XT = sb.tile([F, N], BF16, tag="XT", bufs=1)
            for t in range(NT):
                pt = ps_c.tile([F, 128], BF16, tag="ps_c", bufs=2)
                nc.tensor.transpose(pt, xb[:, t, :], ident)
                nc.scalar.copy(out=XT[:, t * 128:(t + 1) * 128], in_=pt)


            # ================= LAYER 1 (U=128) =================
            inv_s1 = 1.0 / float(np.sqrt(U1))

            QT1 = sb.tile([128, H, N], BF16, tag="QT1", bufs=1)
            KT1 = sb.tile([128, H, N], BF16, tag="KT1", bufs=1)
            qk_done = set()

            def qk_proj1(d):
                if d in qk_done or d >= H:
                    return
                qk_done.add(d)
                for w, dst, on_act in ((wq1, QT1, True), (wk1, KT1, False)):
                    for q in range(2):
                        p = ps_s.tile([128, 512], F32, tag="ps_s", bufs=2)
                        nc.tensor.matmul(p, lhsT=w[:, d * 128:(d + 1) * 128],
                                         rhs=XT[:, q * 512:(q + 1) * 512],
                                         start=True, stop=True)
                        dsl = dst[:, d, q * 512:(q + 1) * 512]
                        if on_act:
                            nc.scalar.copy(out=dsl, in_=p)
                        else:
                            nc.vector.tensor_copy(out=dsl, in_=p)

            # V1 [128, NT, H*U1] fp16 (m-part, (h,u)); c-chunk 0 = heads 0-3
            V1 = sb.tile([128, NT, H * U1], FP16, tag="V1", bufs=1)

            def v_proj1(c):
                for m in range(NT):
                    p = ps_c.tile([128, 512], F32, tag="ps_c", bufs=2,
                                  name="pv1")
                    nc.tensor.matmul(p, lhsT=XT[:, m * 128:(m + 1) * 128],
                                     rhs=wv1[:, c * 512:(c + 1) * 512],
                                     start=True, stop=True)
                    nc.scalar.copy(out=V1[:, m, c * 512:(c + 1) * 512], in_=p)

            qk_proj1(0)
            qk_proj1(1)
            v_proj1(0)

            CT1 = sb.tile([128, H, N], FP16, tag="CT1", bufs=1)
            E_t = {}
            z_t = {}

            def s_block1(h, m):
                """scores + exp + mask + z-partials for (head h, m-chunk)."""
                if m == 0:
                    E_t[h] = sb.tile([128, NT, N], FP16, tag="E", bufs=2,
                                     name=f"E{h}")
                E = E_t[h]
                s = ps_s.tile([128, 1024], F32, tag="ps_s", bufs=2)
                for q in range(2):
                    nc.tensor.matmul(
                        s[:, q * 512:(q + 1) * 512],
                        lhsT=KT1[:, h, m * 128:(m + 1) * 128],
                        rhs=QT1[:, h, q * 512:(q + 1) * 512],
                        start=True, stop=True)
                nc.scalar.activation(out=E[:, m, :], in_=s, func=AF.Exp,
                                     scale=inv_s1)
                nc.vector.tensor_mul(out=E[:, m, :], in0=E[:, m, :],
                                     in1=Mt[:, m, :])
                with nc.allow_low_precision(
                        reason="fp16 softmax-denominator partials; <=1024 "
                               "positive O(1) terms, ~1e-3 rel"):
                    if m == 1:
                        zacc = sb.tile([128, N], FP16, tag="zacc", bufs=2)
                        z_t[h] = zacc
                        nc.vector.tensor_add(out=zacc, in0=E[:, 0, :],
                                             in1=E[:, 1, :])
                    elif m > 1:
                        zacc = z_t[h]
                        nc.vector.tensor_add(out=zacc, in0=zacc,
                                             in1=E[:, m, :])
                    if m == NT - 1:
                        zat = sb.tile([128, NT, 128], FP16, tag="zat", bufs=2)
                        nc.sync.dma_start(out=zat, in_=z_t.pop(h),
                                          transpose=True)
                        zsum = sb.tile([128, 32], F32, tag="zsum", bufs=2)
                        nc.vector.reduce_sum(out=zsum[:, 0:NT].rearrange(
                                                 "p (t x) -> p t x", x=1),
                                             in_=zat,
                                             axis=mybir.AxisListType.X)
                        zit = sb.tile([128, 128], FP16, tag="zit", bufs=2)
                        nc.vector.memset(zit[:, NT:], 0.0)
                        nc.vector.reciprocal(out=zit[:, 0:NT],
                                             in_=zsum[:, 0:NT])
                        zrow = sb.tile([128, 128], FP16, tag="zrow", bufs=2)
                        nc.sync.dma_start(out=zrow, in_=zit, transpose=True)
                        zl = sb.tile([1, N], FP16, tag="zl", bufs=2)
                        nc.sync.dma_start(
                            out=zl.rearrange("x (t c) -> x t c", t=NT),
                            in_=zrow[0:NT, :])
                        zinv = sb.tile([128, N], FP16, tag="zinv", bufs=2)
                        nc.gpsimd.partition_broadcast(out_ap=zinv, in_ap=zl,
                                                      channels=128)
                        z_t[(h, 'inv')] = zinv

            def ctx_block1(h, m, cp):
                E = E_t[h]
                for q in range(2):
                    nc.tensor.matmul(
                        cp[:, q * 512:(q + 1) * 512],
                        lhsT=V1[:, m, h * U1:(h + 1) * U1],
                        rhs=E[:, m, q * 512:(q + 1) * 512],
                        start=(m == 0), stop=(m == NT - 1))
                if m == NT - 1:
                    E_t.pop(h)
                    zinv = z_t.pop((h, 'inv'))
                    for q in range(2):
                        nc.vector.tensor_mul(
                            out=CT1[:, h, q * 512:(q + 1) * 512],
                            in0=cp[:, q * 512:(q + 1) * 512],
                            in1=zinv[:, q * 512:(q + 1) * 512])

            # deferred loads, spread across the layer-1 head loop
            deferred = {}

            def deferred_loads(h):
                if h == 1:
                    v_proj1(1)
                elif h == 2:
                    deferred['wo1'] = load_w("Wo1", w_d["Wo1"], None, U1,
                                             kt=H * U1 // 128)
                elif h == 3:
                    deferred['wq2'] = load_w("Wq2", w_d["Wq2"], U1, H * U2)
                    deferred['wk2'] = load_w("Wk2", w_d["Wk2"], U1, H * U2)
                elif h == 4:
                    deferred['wv2'] = load_w("Wv2", w_d["Wv2"], U1, H * U2)
                    deferred['wo2'] = load_w("Wo2", w_d["Wo2"], None, U2,
                                             kt=H * U2 // 128)
                elif h == 5:
                    for nm, shp in (("W1", [F, 32]), ("W2", [32, 16]),
                                    ("W3", [16, 1])):
                        t = sb.tile(shp, F32, tag=nm, bufs=1, name=nm)
                        nc.sync.dma_start(out=t, in_=w_d[nm][:])
                        deferred[nm] = t
                    for nm, p in (("b1", 32), ("b2", 16), ("b3", 1)):
                        t = sb.tile([p, 1], F32, tag=nm, bufs=1, name=nm)
                        nc.sync.dma_start(
                            out=t, in_=w_d[nm][:].rearrange("(p x) -> p x", x=1))
                        deferred[nm] = t

            for m in range(NT):
                s_block1(0, m)
            for h in range(H):
                qk_proj1(h + 2)
                deferred_loads(h)
                cp = ps_c.tile([128, 1024], F32, tag="ps_c", bufs=2)
                for m in range(NT):
                    if h + 1 < H:
                        s_block1(h + 1, m)
                    ctx_block1(h, m, cp)

            wo1 = deferred['wo1']
            wq2, wk2 = deferred['wq2'], deferred['wk2']
            wv2, wo2 = deferred['wv2'], deferred['wo2']
            w1, w2, w3 = deferred['W1'], deferred['W2'], deferred['W3']
            b1, b2, b3 = deferred['b1'], deferred['b2'], deferred['b3']

            # Wo1: H1T [128, N] fp16 = Wo1^T @ CT1
            H1T = sb.tile([128, N], FP16, tag="H1T", bufs=1)
            for q in range(2):
                wp = ps_s.tile([128, 512], F32, tag="ps_s", bufs=2)
                for k in range(H):
                    nc.tensor.matmul(wp, lhsT=wo1[:, k, :],
                                     rhs=CT1[:, k, q * 512:(q + 1) * 512],
                                     start=(k == 0), stop=(k == H - 1))
                nc.scalar.copy(out=H1T[:, q * 512:(q + 1) * 512], in_=wp)

            # ================= LAYER 2 (U=64) =================
            inv_s2 = 1.0 / float(np.sqrt(U2))
            DT2 = H * U2 // 128  # 4

            QT2 = sb.tile([128, DT2, N], FP16, tag="QT2", bufs=1)
            KT2 = sb.tile([128, DT2, N], FP16, tag="KT2", bufs=1)
            for d in range(DT2):
                for w, dst in ((wq2, QT2), (wk2, KT2)):
                    for q in range(2):
                        p = ps_s.tile([128, 512], F32, tag="ps_s", bufs=2)
                        nc.tensor.matmul(p, lhsT=w[:, d * 128:(d + 1) * 128],
                                         rhs=H1T[:, q * 512:(q + 1) * 512],
                                         start=True, stop=True)
                        nc.scalar.copy(out=dst[:, d, q * 512:(q + 1) * 512],
                                       in_=p)

            # V2 [128, NT, H*U2] fp16
            V2 = sb.tile([128, NT, H * U2], FP16, tag="V2A", bufs=1)
            for m in range(NT):
                p = ps_s.tile([128, 512], F32, tag="ps_s", bufs=2)
                nc.tensor.matmul(p, lhsT=H1T[:, m * 128:(m + 1) * 128],
                                 rhs=wv2, start=True, stop=True)
                nc.scalar.copy(out=V2[:, m, :], in_=p)

            CT2 = sb.tile([128, DT2, N], FP16, tag="CT2", bufs=1)

            def s_block2(h, m):
                if m == 0:
                    E_t[h] = sb.tile([128, NT, N], FP16, tag="E", bufs=2,
                                     name=f"E{h}")
                E = E_t[h]
                r0 = (h % 2) * U2
                kt = h // 2
                s = ps_s.tile([128, 1024], F32, tag="ps_s", bufs=2)
                for q in range(2):
                    nc.tensor.matmul(
                        s[:, q * 512:(q + 1) * 512],
                        lhsT=KT2[r0:r0 + U2, kt, m * 128:(m + 1) * 128],
                        rhs=QT2[r0:r0 + U2, kt, q * 512:(q + 1) * 512],
                        start=True, stop=True)
                nc.scalar.activation(out=E[:, m, :], in_=s, func=AF.Exp,
                                     scale=inv_s2)
                nc.vector.tensor_mul(out=E[:, m, :], in0=E[:, m, :],
                                     in1=Mt[:, m, :])
                with nc.allow_low_precision(
                        reason="fp16 softmax-denominator partials"):
                    if m == 1:
                        zacc = sb.tile([128, N], FP16, tag="zacc", bufs=2)
                        z_t[h] = zacc
                        nc.vector.tensor_add(out=zacc, in0=E[:, 0, :],
                                             in1=E[:, 1, :])
                    elif m > 1:
                        nc.vector.tensor_add(out=z_t[h], in0=z_t[h],
                                             in1=E[:, m, :])
                    if m == NT - 1:
                        zat = sb.tile([128, NT, 128], FP16, tag="zat", bufs=2)
                        nc.sync.dma_start(out=zat, in_=z_t.pop(h),
                                          transpose=True)
                        zsum = sb.tile([128, 32], F32, tag="zsum", bufs=2)
                        nc.vector.reduce_sum(out=zsum[:, 0:NT].rearrange(
                                                 "p (t x) -> p t x", x=1),
                                             in_=zat,
                                             axis=mybir.AxisListType.X)
                        zit = sb.tile([128, 128], FP16, tag="zit", bufs=2)
                        nc.vector.memset(zit[:, NT:], 0.0)
                        nc.vector.reciprocal(out=zit[:, 0:NT],
                                             in_=zsum[:, 0:NT])
                        zrow = sb.tile([128, 128], FP16, tag="zrow", bufs=2)
                        nc.sync.dma_start(out=zrow, in_=zit, transpose=True)
                        zl = sb.tile([1, N], FP16, tag="zl", bufs=2)
                        nc.sync.dma_start(
                            out=zl.rearrange("x (t c) -> x t c", t=NT),
                            in_=zrow[0:NT, :])
                        zinv = sb.tile([128, N], FP16, tag="zinv", bufs=2)
                        nc.gpsimd.partition_broadcast(out_ap=zinv, in_ap=zl,
                                                      channels=128)
                        z_t[(h, 'inv')] = zinv

            def ctx_block2(h, m, cp):
                E = E_t[h]
                for q in range(2):
                    nc.tensor.matmul(
                        cp[0:U2, q * 512:(q + 1) * 512],
                        lhsT=V2[:, m, h * U2:(h + 1) * U2],
                        rhs=E[:, m, q * 512:(q + 1) * 512],
                        start=(m == 0), stop=(m == NT - 1))
                if m == NT - 1:
                    E_t.pop(h)
                    zinv = z_t.pop((h, 'inv'))
                    r0 = (h % 2) * U2
                    kt = h // 2
                    for q in range(2):
                        nc.vector.tensor_mul(
                            out=CT2[r0:r0 + U2, kt, q * 512:(q + 1) * 512],
                            in0=cp[0:U2, q * 512:(q + 1) * 512],
                            in1=zinv[0:U2, q * 512:(q + 1) * 512])

            for m in range(NT):
                s_block2(0, m)
            for h in range(H):
                cp = ps_c.tile([128, 1024], F32, tag="ps_c", bufs=2)
                for m in range(NT):
                    if h + 1 < H:
                        s_block2(h + 1, m)
                    ctx_block2(h, m, cp)

            # ---- mean pool (over n) then Wo2 fold + MLP ----
            pooled = sb.tile([128, DT2], F32, tag="pooled", bufs=1)
            for k in range(DT2):
                nc.vector.reduce_sum(out=pooled[:, k:k + 1], in_=CT2[:, k, :],
                                     axis=mybir.AxisListType.X)
            pooledh = sb.tile([128, DT2], FP16, tag="pooledh", bufs=1)
            nc.vector.tensor_copy(out=pooledh, in_=pooled)
            p0 = ps_c.tile([U2, 1], F32, tag="ps_c", bufs=2)
            for k in range(DT2):
                nc.tensor.matmul(p0, lhsT=wo2[:, k, :], rhs=pooledh[:, k:k + 1],
                                 start=(k == 0), stop=(k == DT2 - 1))
            hs = sb.tile([U2, 1], F32, tag="hs", bufs=1)
            nc.scalar.copy(out=hs, in_=p0)
            p1 = ps_c.tile([32, 1], F32, tag="ps_c", bufs=2)
            nc.tensor.matmul(p1, lhsT=w1, rhs=hs, start=True, stop=True)
            a1 = sb.tile([32, 1], F32, tag="a1", bufs=1)
            nc.scalar.activation(out=a1, in_=p1, func=AF.Relu, bias=b1,
                                 scale=1.0 / float(N))
            p2 = ps_c.tile([16, 1], F32, tag="ps_c", bufs=2)
            nc.tensor.matmul(p2, lhsT=w2, rhs=a1, start=True, stop=True)
            a2 = sb.tile([16, 1], F32, tag="a2", bufs=1)
            nc.scalar.activation(out=a2, in_=p2, func=AF.Relu, bias=b2)
            p3 = ps_c.tile([1, 1], F32, tag="ps_c", bufs=2)
            nc.tensor.matmul(p3, lhsT=w3, rhs=a2, start=True, stop=True)
            yt = sb.tile([1, 1], F32, tag="yt", bufs=1)
            nc.vector.tensor_add(out=yt, in0=p3, in1=b3)
            nc.sync.dma_start(out=y_d[:], in_=yt)

    nc.compile()
    return nc


_NC = None


def _get_nc():
    global _NC
    if _NC is None:
        _NC = build_nc()
    return _NC


def make_in_maps(inputs):
    in_maps = []
    for i in range(B):
        m = {"X": np.ascontiguousarray(np.asarray(inputs["X"][i], dtype=np.float32)),
             "A": np.ascontiguousarray(np.asarray(inputs["A"][i], dtype=np.float32))}
        for k in WEIGHT_NAMES:
            m[k] = np.ascontiguousarray(np.asarray(inputs[k], dtype=np.float32))
        in_maps.append(m)
    return in_maps


def run(inputs, trace=False):
    nc = _get_nc()
    res = run_bass_kernel_spmd(nc, make_in_maps(inputs), list(range(B)), trace=trace)
    y = np.stack([res.results[i]["y"][0] for i in range(B)], axis=0)
    return y.astype(np.float32), res


def kernel(**inputs):
    y, _ = run(inputs, trace=False)
    return y


# revision 20
# speedup vs baseline: 1.1527x; 1.0350x over previous
"""Trainium2 Bass kernel for nn_CustomGNN_66881230733874 (2-layer GAT + mean-pool + MLP).

Sharding: data-parallel over batch B=8 -> one graph per NeuronCore (8 cores).
Each core computes its full graph end-to-end (no collectives); host gathers [8,1].

V2.1 design notes:
  - ctx computed TRANSPOSED (ctx^T[u, n]) with V as the stationary matmul
    operand and E streaming as wide rhs: no per-step LDWEIGHTS reload of E
    chunks, no SBUF DMA-transposes of ctx.
  - whole datapath in fp16 (11-bit mantissa): single-matmul V and Wo paths
    (no split-hi/lo), Q/K/scores in fp16 as well.
  - layer1 softmax denominator: running sum of E tiles (DVE + gpsimd mix) +
    gpsimd partition_all_reduce, reciprocal_approx_fast (f32), normalize
    fused into the PSUM->SBUF drain.
  - layer2 denominator: ones-column in V_aug (65-wide lhsT), reciprocal of
    PSUM row 64, gpsimd partition_broadcast, fused normalize.
  - Wo2 + mean-pool folded after ctx2^T free-axis reduction.
  - prelude: A-mask pipeline and first head's scores start immediately;
    weight loads and projections are interleaved into the head loop.
"""

import numpy as np

import concourse.bass as bass
import concourse.mybir as mybir
import concourse.tile as tile
from concourse import bacc
from concourse import bass_isa
from concourse.bass_utils import run_bass_kernel_spmd
from concourse.masks import make_identity

F32 = mybir.dt.float32
BF16 = mybir.dt.bfloat16
FP16 = mybir.dt.float16
AF = mybir.ActivationFunctionType
OP = mybir.AluOpType

B = 8
N = 1024
F = 64
H = 8
U1, U2 = 128, 64
NT = N // 128  # 8 node chunks

WEIGHT_NAMES = [
    "Wq1", "Wk1", "Wv1", "Wo1", "Wq2", "Wk2", "Wv2", "Wo2",
    "W1", "b1", "W2", "b2", "W3", "b3",
]


def build_nc(repeats=1):
    nc = bacc.Bacc("TRN2", target_bir_lowering=False, debug=False)

    x_d = nc.dram_tensor("X", [N, F], F32, kind="ExternalInput")
    a_d = nc.dram_tensor("A", [N, N], F32, kind="ExternalInput")
    w_d = {}
    shapes = {
        "Wq1": [F, H * U1], "Wk1": [F, H * U1], "Wv1": [F, H * U1],
        "Wo1": [H * U1, U1],
        "Wq2": [U1, H * U2], "Wk2": [U1, H * U2], "Wv2": [U1, H * U2],
        "Wo2": [H * U2, U2],
        "W1": [F, 32], "b1": [32], "W2": [32, 16], "b2": [16],
        "W3": [16, 1], "b3": [1],
    }
    for k, s in shapes.items():
        w_d[k] = nc.dram_tensor(k, s, F32, kind="ExternalInput")
    y_d = nc.dram_tensor("y", [1, 1], F32, kind="ExternalOutput")

    with tile.TileContext(nc) as tc:
        with (
            tc.tile_pool(name="sb", bufs=1) as sb,
            tc.tile_pool(name="scratch", bufs=2) as scratch,
            tc.tile_pool(name="ps_s", bufs=2, space="PSUM") as ps_s,
            tc.tile_pool(name="ps_c", bufs=2, space="PSUM") as ps_c,
        ):
          for _rep in range(repeats):

            def load_w(name, dram, part, cols, kt=None, dt=FP16):
                """DMA f32 weight -> SBUF fp16 (optionally [128, kt, cols])."""
                if kt is None:
                    w32 = scratch.tile([part, cols], F32, tag=(f"ws_{name}" if name in ("Wq1", "Wk1") else "wscratch"),
                                       bufs=1 if name in ("Wq1", "Wk1") else 2,
                                       name=f"w32_{name}")
                    nc.sync.dma_start(out=w32, in_=dram[:])
                    hi = sb.tile([part, cols], dt, tag=f"w_{name}", bufs=1,
                                 name=f"w_{name}")
                else:
                    w32 = scratch.tile([128, kt, cols], F32, tag="wscratch",
                                       bufs=2, name=f"w32_{name}")

                    nc.sync.dma_start(
                        out=w32, in_=dram[:].rearrange("(k p) c -> p k c", p=128))
                    hi = sb.tile([128, kt, cols], dt, tag=f"w_{name}", bufs=1,
                                 name=f"w_{name}")
                nc.vector.tensor_copy(out=hi, in_=w32)
                return hi

            ident = sb.tile([128, 128], BF16, tag="ident", bufs=1)
            make_identity(nc, ident)
            x32 = scratch.tile([128, NT, F], F32, tag="x32", bufs=1, name="x32")
            nc.sync.dma_start(out=x32, in_=x_d[:].rearrange("(t p) f -> p t f", p=128))
            wq1 = load_w("Wq1", w_d["Wq1"], F, H * U1, dt=BF16)
            wk1 = load_w("Wk1", w_d["Wk1"], F, H * U1, dt=BF16)
            wv1 = load_w("Wv1", w_d["Wv1"], F, H * U1, dt=BF16)

            # ---- A-mask pipeline (longest prelude chain) ----
            mn = sb.tile([128, NT, N], FP16, tag="E", bufs=2)
            for cc in range(NT):
                a32c = scratch.tile([128, N], F32, tag="a32c", bufs=2)
                nc.sync.dma_start(
                    out=a32c,
                    in_=a_d[:].rearrange("(t p) n -> p t n", p=128)[:, cc, :])
                nc.vector.tensor_scalar(out=mn[:, cc, :], in0=a32c, scalar1=-1.0,
                                        scalar2=1.0, op0=OP.mult, op1=OP.add)
            Mt = sb.tile([128, NT, N], FP16, tag="Mt", bufs=1)
            for cc in range(NT):
                nc.sync.dma_start(out=Mt[:, :, cc * 128:(cc + 1) * 128],
                                  in_=mn[:, cc, :], transpose=True)

            # ---- XT [F, N] bf16 via PE transpose ----

            xb = sb.tile([128, NT, F], BF16, tag="xb", bufs=1)
            nc.vector.tensor_copy(out=xb, in_=x32)
            XT = sb.tile([F, N], BF16, tag="XT", bufs=1)
            for t in range(NT):
                pt = ps_c.tile([F, 128], BF16, tag="ps_c", bufs=2)
                nc.tensor.transpose(pt, xb[:, t, :], ident)
                nc.scalar.copy(out=XT[:, t * 128:(t + 1) * 128], in_=pt)


            # ================= LAYER 1 (U=128) =================
            inv_s1 = 1.0 / float(np.sqrt(U1))

            QT1 = sb.tile([128, H, N], BF16, tag="QT1", bufs=1)
            KT1 = sb.tile([128, H, N], BF16, tag="KT1", bufs=1)
            qk_done = set()

            def qk_proj1(d):
                if d in qk_done or d >= H:
                    return
                qk_done.add(d)
                for w, dst, on_act in ((wq1, QT1, True), (wk1, KT1, False)):
                    for q in range(2):
                        p = ps_s.tile([128, 512], F32, tag="ps_s", bufs=2)
                        nc.tensor.matmul(p, lhsT=w[:, d * 128:(d + 1) * 128],
                                         rhs=XT[:, q * 512:(q + 1) * 512],
                                         start=True, stop=True)
                        dsl = dst[:, d, q * 512:(q + 1) * 512]
                        if on_act:
                            nc.scalar.copy(out=dsl, in_=p)
                        else:
                            nc.vector.tensor_copy(out=dsl, in_=p)

            # V1 [128, NT, H*U1] fp16 (m-part, (h,u)); c-chunk 0 = heads 0-3
            V1 = sb.tile([128, NT, H * U1], FP16, tag="V1", bufs=1)

            def v_proj1(c):
                for m in range(NT):
                    p = ps_c.tile([128, 512], F32, tag="ps_c", bufs=2,
                                  name="pv1")
                    nc.tensor.matmul(p, lhsT=XT[:, m * 128:(m + 1) * 128],
                                     rhs=wv1[:, c * 512:(c + 1) * 512],
                                     start=True, stop=True)
                    nc.scalar.copy(out=V1[:, m, c * 512:(c + 1) * 512], in_=p)

            qk_proj1(0)
            qk_proj1(1)
            v_proj1(0)

            CT1 = sb.tile([128, H, N], FP16, tag="CT1", bufs=1)
            E_t = {}
            z_t = {}

            def s_block1(h, m):
                """scores + exp + mask + z-partials for (head h, m-chunk)."""
                if m == 0:
                    E_t[h] = sb.tile([128, NT, N], FP16, tag="E", bufs=2,
                                     name=f"E{h}")
                E = E_t[h]
                s = ps_s.tile([128, 1024], F32, tag="ps_s", bufs=2)
                for q in range(2):
                    nc.tensor.matmul(
                        s[:, q * 512:(q + 1) * 512],
                        lhsT=KT1[:, h, m * 128:(m + 1) * 128],
                        rhs=QT1[:, h, q * 512:(q + 1) * 512],
                        start=True, stop=True)
                nc.scalar.activation(out=E[:, m, :], in_=s, func=AF.Exp,
                                     scale=inv_s1)
                nc.vector.tensor_mul(out=E[:, m, :], in0=E[:, m, :],
                                     in1=Mt[:, m, :])
                with nc.allow_low_precision(
                        reason="fp16 softmax-denominator partials; <=1024 "
                               "positive O(1) terms, ~1e-3 rel"):
                    if m == 1:
                        zacc = sb.tile([128, N], FP16, tag="zacc", bufs=2)
                        z_t[h] = zacc
                        nc.vector.tensor_add(out=zacc, in0=E[:, 0, :],
                                             in1=E[:, 1, :])
                    elif m > 1:
                        zacc = z_t[h]
                        nc.vector.tensor_add(out=zacc, in0=zacc,
                                             in1=E[:, m, :])
                    if m == NT - 1:
                        zat = sb.tile([128, NT, 128], FP16, tag="zat", bufs=2)
                        nc.sync.dma_start(out=zat, in_=z_t.pop(h),
                                          transpose=True)
                        zsum = sb.tile([128, 32], F32, tag="zsum", bufs=2)
                        nc.vector.reduce_sum(out=zsum[:, 0:NT].rearrange(
                                                 "p (t x) -> p t x", x=1),
                                             in_=zat,
                                             axis=mybir.AxisListType.X)
                        zit = sb.tile([128, 128], FP16, tag="zit", bufs=2)
                        nc.vector.memset(zit[:, NT:], 0.0)
                        nc.vector.reciprocal(out=zit[:, 0:NT],
                                             in_=zsum[:, 0:NT])
                        zrow = sb.tile([128, 128], FP16, tag="zrow", bufs=2)
                        nc.sync.dma_start(out=zrow, in_=zit, transpose=True)
                        zl = sb.tile([1, N], FP16, tag="zl", bufs=2)
                        nc.sync.dma_start(
                            out=zl.rearrange("x (t c) -> x t c", t=NT),
                            in_=zrow[0:NT, :])
                        zinv = sb.tile([128, N], FP16, tag="zinv", bufs=2)
                        nc.gpsimd.partition_broadcast(out_ap=zinv, in_ap=zl,
                                                      channels=128)
                        z_t[(h, 'inv')] = zinv

            def ctx_block1(h, m, cp):
                E = E_t[h]
                for q in range(2):
                    nc.tensor.matmul(
                        cp[:, q * 512:(q + 1) * 512],
                        lhsT=V1[:, m, h * U1:(h + 1) * U1],
                        rhs=E[:, m, q * 512:(q + 1) * 512],
                        start=(m == 0), stop=(m == NT - 1))
                if m == NT - 1:
                    E_t.pop(h)
                    zinv = z_t.pop((h, 'inv'))
                    for q in range(2):
                        nc.vector.tensor_mul(
                            out=CT1[:, h, q * 512:(q + 1) * 512],
                            in0=cp[:, q * 512:(q + 1) * 512],
                            in1=zinv[:, q * 512:(q + 1) * 512])

            # deferred loads, spread across the layer-1 head loop
            deferred = {}

            def deferred_loads(h):
                if h == 1:
                    v_proj1(1)
                elif h == 2:
                    deferred['wo1'] = load_w("Wo1", w_d["Wo1"], None, U1,
                                             kt=H * U1 // 128)
                elif h == 3:
                    deferred['wq2'] = load_w("Wq2", w_d["Wq2"], U1, H * U2)
                    deferred['wk2'] = load_w("Wk2", w_d["Wk2"], U1, H * U2)
                elif h == 4:
                    deferred['wv2'] = load_w("Wv2", w_d["Wv2"], U1, H * U2)
                    deferred['wo2'] = load_w("Wo2", w_d["Wo2"], None, U2,
                                             kt=H * U2 // 128)
                elif h == 5:
                    for nm, shp in (("W1", [F, 32]), ("W2", [32, 16]),
                                    ("W3", [16, 1])):
                        t = sb.tile(shp, F32, tag=nm, bufs=1, name=nm)
                        nc.sync.dma_start(out=t, in_=w_d[nm][:])
                        deferred[nm] = t
                    for nm, p in (("b1", 32), ("b2", 16), ("b3", 1)):
                        t = sb.tile([p, 1], F32, tag=nm, bufs=1, name=nm)
                        nc.sync.dma_start(
                            out=t, in_=w_d[nm][:].rearrange("(p x) -> p x", x=1))
                        deferred[nm] = t

            for m in range(NT):
                s_block1(0, m)
            for h in range(H):
                qk_proj1(h + 2)
                deferred_loads(h)
                cp = ps_c.tile([128, 1024], F32, tag="ps_c", bufs=2)
                for m in range(NT):
                    if h + 1 < H:
                        s_block1(h + 1, m)
                    ctx_block1(h, m, cp)

            wo1 = deferred['wo1']
            wq2, wk2 = deferred['wq2'], deferred['wk2']
            wv2, wo2 = deferred['wv2'], deferred['wo2']
            w1, w2, w3 = deferred['W1'], deferred['W2'], deferred['W3']
            b1, b2, b3 = deferred['b1'], deferred['b2'], deferred['b3']

            # Wo1: H1T [128, N] fp16 = Wo1^T @ CT1
            H1T = sb.tile([128, N], FP16, tag="H1T", bufs=1)
            for q in range(2):
                wp = ps_s.tile([128, 512], F32, tag="ps_s", bufs=2)
                for k in range(H):
                    nc.tensor.matmul(wp, lhsT=wo1[:, k, :],
                                     rhs=CT1[:, k, q * 512:(q + 1) * 512],
                                     start=(k == 0), stop=(k == H - 1))
                nc.scalar.copy(out=H1T[:, q * 512:(q + 1) * 512], in_=wp)

            # ================= LAYER 2 (U=64) =================
            inv_s2 = 1.0 / float(np.sqrt(U2))
            DT2 = H * U2 // 128  # 4

            QT2 = sb.tile([128, DT2, N], FP16, tag="QT2", bufs=1)
            KT2 = sb.tile([128, DT2, N], FP16, tag="KT2", bufs=1)
            for d in range(DT2):
                for w, dst in ((wq2, QT2), (wk2, KT2)):
                    for q in range(2):
                        p = ps_s.tile([128, 512], F32, tag="ps_s", bufs=2)
                        nc.tensor.matmul(p, lhsT=w[:, d * 128:(d + 1) * 128],
                                         rhs=H1T[:, q * 512:(q + 1) * 512],
                                         start=True, stop=True)
                        nc.scalar.copy(out=dst[:, d, q * 512:(q + 1) * 512],
                                       in_=p)

            # V2 [128, NT, H*U2] fp16
            V2 = sb.tile([128, NT, H * U2], FP16, tag="V2A", bufs=1)
            for m in range(NT):
                p = ps_s.tile([128, 512], F32, tag="ps_s", bufs=2)
                nc.tensor.matmul(p, lhsT=H1T[:, m * 128:(m + 1) * 128],
                                 rhs=wv2, start=True, stop=True)
                nc.scalar.copy(out=V2[:, m, :], in_=p)

            CT2 = sb.tile([128, DT2, N], FP16, tag="CT2", bufs=1)

            def s_block2(h, m):
                if m == 0:
                    E_t[h] = sb.tile([128, NT, N], FP16, tag="E", bufs=2,
                                     name=f"E{h}")
                E = E_t[h]
                r0 = (h % 2) * U2
                kt = h // 2
                s = ps_s.tile([128, 1024], F32, tag="ps_s", bufs=2)
                for q in range(2):
                    nc.tensor.matmul(
                        s[:, q * 512:(q + 1) * 512],
                        lhsT=KT2[r0:r0 + U2, kt, m * 128:(m + 1) * 128],
                        rhs=QT2[r0:r0 + U2, kt, q * 512:(q + 1) * 512],
                        start=True, stop=True)
                nc.scalar.activation(out=E[:, m, :], in_=s, func=AF.Exp,
                                     scale=inv_s2)
                nc.vector.tensor_mul(out=E[:, m, :], in0=E[:, m, :],
                                     in1=Mt[:, m, :])
                with nc.allow_low_precision(
                        reason="fp16 softmax-denominator partials"):
                    if m == 1:
                        zacc = sb.tile([128, N], FP16, tag="zacc", bufs=2)
                        z_t[h] = zacc
                        nc.vector.tensor_add(out=zacc, in0=E[:, 0, :],
                                             in1=E[:, 1, :])
                    elif m > 1:
                        nc.vector.tensor_add(out=z_t[h], in0=z_t[h],
                                             in1=E[:, m, :])
                    if m == NT - 1:
                        zat = sb.tile([128, NT, 128], FP16, tag="zat", bufs=2)
                        nc.sync.dma_start(out=zat, in_=z_t.pop(h),
                                          transpose=True)
                        zsum = sb.tile([128, 32], F32, tag="zsum", bufs=2)
                        nc.vector.reduce_sum(out=zsum[:, 0:NT].rearrange(
                                                 "p (t x) -> p t x", x=1),
                                             in_=zat,
                                             axis=mybir.AxisListType.X)
                        zit = sb.tile([128, 128], FP16, tag="zit", bufs=2)
                        nc.vector.memset(zit[:, NT:], 0.0)
                        nc.vector.reciprocal(out=zit[:, 0:NT],
                                             in_=zsum[:, 0:NT])
                        zrow = sb.tile([128, 128], FP16, tag="zrow", bufs=2)
                        nc.sync.dma_start(out=zrow, in_=zit, transpose=True)
                        zl = sb.tile([1, N], FP16, tag="zl", bufs=2)
                        nc.sync.dma_start(
                            out=zl.rearrange("x (t c) -> x t c", t=NT),
                            in_=zrow[0:NT, :])
                        zinv = sb.tile([128, N], FP16, tag="zinv", bufs=2)
                        nc.gpsimd.partition_broadcast(out_ap=zinv, in_ap=zl,
                                                      channels=128)
                        z_t[(h, 'inv')] = zinv

            def ctx_block2(h, m, cp):
                E = E_t[h]
                for q in range(2):
                    nc.tensor.matmul(
                        cp[0:U2, q * 512:(q + 1) * 512],
                        lhsT=V2[:, m, h * U2:(h + 1) * U2],
                        rhs=E[:, m, q * 512:(q + 1) * 512],
                        start=(m == 0), stop=(m == NT - 1))
                if m == NT - 1:
                    E_t.pop(h)
                    zinv = z_t.pop((h, 'inv'))
                    r0 = (h % 2) * U2
                    kt = h // 2
                    for q in range(2):
                        nc.vector.tensor_mul(
                            out=CT2[r0:r0 + U2, kt, q * 512:(q + 1) * 512],
                            in0=cp[0:U2, q * 512:(q + 1) * 512],
                            in1=zinv[0:U2, q * 512:(q + 1) * 512])

            for m in range(NT):
                s_block2(0, m)
            for h in range(H):
                cp = ps_c.tile([128, 1024], F32, tag="ps_c", bufs=2)
                for m in range(NT):
                    if h + 1 < H:
                        s_block2(h + 1, m)
                    ctx_block2(h, m, cp)

            # ---- mean pool (over n) then Wo2 fold + MLP ----
            pooled = sb.tile([128, DT2], F32, tag="pooled", bufs=1)
            for k in range(DT2):
                nc.vector.reduce_sum(out=pooled[:, k:k + 1], in_=CT2[:, k, :],
                                     axis=mybir.AxisListType.X)
            pooledh = sb.tile([128, DT2], FP16, tag="pooledh", bufs=1)
            nc.vector.tensor_copy(out=pooledh, in_=pooled)
            p0 = ps_c.tile([U2, 1], F32, tag="ps_c", bufs=2)
            for k in range(DT2):
                nc.tensor.matmul(p0, lhsT=wo2[:, k, :], rhs=pooledh[:, k:k + 1],
                                 start=(k == 0), stop=(k == DT2 - 1))
            hs = sb.tile([U2, 1], F32, tag="hs", bufs=1)
            nc.scalar.copy(out=hs, in_=p0)
            p1 = ps_c.tile([32, 1], F32, tag="ps_c", bufs=2)
            nc.tensor.matmul(p1, lhsT=w1, rhs=hs, start=True, stop=True)
            a1 = sb.tile([32, 1], F32, tag="a1", bufs=1)
            nc.scalar.activation(out=a1, in_=p1, func=AF.Relu, bias=b1,
                                 scale=1.0 / float(N))
            p2 = ps_c.tile([16, 1], F32, tag="ps_c", bufs=2)
            nc.tensor.matmul(p2, lhsT=w2, rhs=a1, start=True, stop=True)
            a2 = sb.tile([16, 1], F32, tag="a2", bufs=1)
            nc.scalar.activation(out=a2, in_=p2, func=AF.Relu, bias=b2)
            p3 = ps_c.tile([1, 1], F32, tag="ps_c", bufs=2)
            nc.tensor.matmul(p3, lhsT=w3, rhs=a2, start=True, stop=True)
            yt = sb.tile([1, 1], F32, tag="yt", bufs=1)
            nc.vector.tensor_add(out=yt, in0=p3, in1=b3)
            nc.sync.dma_start(out=y_d[:], in_=yt)

    nc.compile()
    return nc


_NC = None


def _get_nc():
    global _NC
    if _NC is None:
        _NC = build_nc()
    return _NC


def make_in_maps(inputs):
    in_maps = []
    for i in range(B):
        m = {"X": np.ascontiguousarray(np.asarray(inputs["X"][i], dtype=np.float32)),
             "A": np.ascontiguousarray(np.asarray(inputs["A"][i], dtype=np.float32))}
        for k in WEIGHT_NAMES:
            m[k] = np.ascontiguousarray(np.asarray(inputs[k], dtype=np.float32))
        in_maps.append(m)
    return in_maps


def run(inputs, trace=False):
    nc = _get_nc()
    res = run_bass_kernel_spmd(nc, make_in_maps(inputs), list(range(B)), trace=trace)
    y = np.stack([res.results[i]["y"][0] for i in range(B)], axis=0)
    return y.astype(np.float32), res


def kernel(**inputs):
    y, _ = run(inputs, trace=False)
    return y


# revision 21
# speedup vs baseline: 1.2795x; 1.1099x over previous
"""Trainium2 Bass kernel for nn_CustomGNN_66881230733874 (2-layer GAT + mean-pool + MLP).

Sharding: data-parallel over batch B=8 -> one graph per NeuronCore (8 cores).
Each core computes its full graph end-to-end (no collectives); host gathers [8,1].

Layout strategy (per core):
  - Activations live feature-on-partition ("transposed"): XT [F, N], QT/KT [(h,d), N].
  - Scores computed directly transposed: S^T[m, n] = K_h Q_h^T via PE.
  - softmax without max-subtraction (scores are O(1) by construction);
    masked entries are exactly zeroed by multiplying exp(S) with (1-A)^T.
  - ctx_unnorm [n, u] via lhsT=E[m,n-chunk], rhs=V_aug[m, u+1] where V_aug has a
    ones column -> last PSUM column accumulates the softmax denominator Z[n].
  - normalize with per-partition reciprocal (DVE), pack bf16, DMA-transpose
    (SBUF xbar) to get ctx^T [(h,u), N] for the W_out matmul. No PE transposes.
  - Weight matmuls use split-bf16 (hi+lo) for the value path (Wv, Wo) to kill
    correlated rounding error; Wq/Wk single-bf16 (score path is tolerant).
"""

import numpy as np

import concourse.bass as bass
import concourse.mybir as mybir
import concourse.tile as tile
from concourse import bacc
from concourse.bass_utils import run_bass_kernel_spmd
from concourse.masks import make_identity

F32 = mybir.dt.float32
BF16 = mybir.dt.bfloat16
AF = mybir.ActivationFunctionType
OP = mybir.AluOpType

B = 8
N = 1024
F = 64
H = 8
U1, U2 = 128, 64
NT = N // 128  # 8 node chunks

WEIGHT_NAMES = [
    "Wq1", "Wk1", "Wv1", "Wo1", "Wq2", "Wk2", "Wv2", "Wo2",
    "W1", "b1", "W2", "b2", "W3", "b3",
]


def _load_weight2d(nc, sb, scratch, name, dram, part, cols, split):
    """DMA a [part, cols] f32 weight to SBUF bf16 (hi[, lo])."""
    w32 = scratch.tile([part, cols], F32, tag="wscratch", bufs=2)
    nc.sync.dma_start(out=w32, in_=dram[:])
    hi = sb.tile([part, cols], BF16, tag=f"w_{name}_hi", bufs=1)
    nc.vector.tensor_copy(out=hi, in_=w32)
    lo = None
    if split:
        lo = sb.tile([part, cols], BF16, tag=f"w_{name}_lo", bufs=1)
        nc.vector.tensor_sub(out=lo, in0=w32, in1=hi)
    return hi, lo


def _load_weight_kt(nc, sb, scratch, name, dram, kt, cols, split):
    """DMA a [kt*128, cols] f32 weight to SBUF bf16 [128, kt, cols] (hi, lo)."""
    w32 = scratch.tile([128, kt, cols], F32, tag="wscratch", bufs=2)
    nc.sync.dma_start(out=w32, in_=dram[:].rearrange("(k p) c -> p k c", p=128))
    hi = sb.tile([128, kt, cols], BF16, tag=f"w_{name}_hi", bufs=1)
    nc.vector.tensor_copy(out=hi, in_=w32)
    lo = sb.tile([128, kt, cols], BF16, tag=f"w_{name}_lo", bufs=1)
    nc.vector.tensor_sub(out=lo, in0=w32, in1=hi)
    return hi, lo


def _gat_layer(nc, sb, ps_big, ps_small, XT, U, Uout,
               Wq, Wk, Wv_hi, Wv_lo, Wo_hi, Wo_lo, Mt, HT_out):
    """One GAT layer. XT: bf16 [Cin, N]. Writes HT_out: bf16 [Uout, N]."""
    HU = H * U
    DT = HU // 128          # number of 128-row tiles of QT/KT
    UD = 128 // U           # heads per QT/KT tile
    VW = U + 1              # V_aug width per head
    inv_sqrt_u = 1.0 / float(np.sqrt(U))

    # ---- projections: QT/KT [(h,d), N] bf16; V_aug [n, H*(U+1)] bf16 ----
    QT = sb.tile([128, DT, N], BF16, tag="QT", bufs=1)
    KT = sb.tile([128, DT, N], BF16, tag="KT", bufs=1)
    VA = sb.tile([128, NT, H * VW], BF16, tag="VA", bufs=1)

    def qk_proj(d, tag="ps_p", pool=None):
        for w, dst, on_act in ((Wq, QT, True), (Wk, KT, False)):
            for q in range(2):
                p = (pool or ps_big).tile([128, 512], F32, tag=tag, bufs=2)
                nc.tensor.matmul(p, lhsT=w[:, d * 128:(d + 1) * 128],
                                 rhs=XT[:, q * 512:(q + 1) * 512],
                                 start=True, stop=True)
                if on_act:
                    nc.scalar.copy(out=dst[:, d, q * 512:(q + 1) * 512], in_=p)
                else:
                    nc.vector.tensor_copy(out=dst[:, d, q * 512:(q + 1) * 512],
                                          in_=p)

    def v_proj():
        n_vc = HU // 512  # 512-wide chunks of H*U
        hpc = 512 // U    # heads per 512 chunk
        for m in range(NT):
            VAm = VA[:, m, :].rearrange("p (h x) -> p h x", h=H)
            nc.vector.memset(VAm[:, :, U:U + 1], 1.0)
        for c in range(n_vc):
            for m in range(NT):
                VAm = VA[:, m, :].rearrange("p (h x) -> p h x", h=H)
                p = ps_big.tile([128, 512], F32, tag="ps_p", bufs=2)
                xm = XT[:, m * 128:(m + 1) * 128]
                nc.tensor.matmul(p, lhsT=xm, rhs=Wv_hi[:, c * 512:(c + 1) * 512],
                                 start=True, stop=False)
                nc.tensor.matmul(p, lhsT=xm, rhs=Wv_lo[:, c * 512:(c + 1) * 512],
                                 start=False, stop=True)
                nc.vector.tensor_copy(
                    out=VAm[:, c * hpc:(c + 1) * hpc, 0:U],
                    in_=p.rearrange("p (h x) -> p h x", h=hpc),
                )

    # ---- attention, software-pipelined: S(h+1) emitted before ctx(h) so the
    # exp stream on ACT is never starved by PE's ctx block. m-chunks are
    # processed in pairs (FD=2048) to halve per-instruction overheads ----
    CT = sb.tile([128, DT, N], BF16, tag="CT", bufs=1)  # ctx^T [(h,u), N]
    E_tiles = {}
    cp2_tiles = {}
    wo_psum = [None, None]
    KD = HU // 128

    def s_phase(h):
        kt_i, k_off = h // UD, (h % UD) * U
        E = sb.tile([128, NT, N], BF16, tag="E", bufs=4)
        E_tiles[h] = E
        for m in range(NT):
            s = ps_big.tile([128, 1024], F32, tag="ps_s", bufs=2)
            for q in range(2):
                nc.tensor.matmul(
                    s[:, q * 512:(q + 1) * 512],
                    lhsT=KT[k_off:k_off + U, kt_i, m * 128:(m + 1) * 128],
                    rhs=QT[k_off:k_off + U, kt_i, q * 512:(q + 1) * 512],
                    start=True, stop=True,
                )
            nc.scalar.activation(out=E[:, m, :], in_=s, func=AF.Exp,
                                 scale=inv_sqrt_u)
            nc.vector.tensor_mul(out=E[:, m, :], in0=E[:, m, :], in1=Mt[:, m, :])

    GH = 2                    # heads per transpose/Wo group
    TPG = GH * U // 128       # CT tiles per group

    def ctx_phase(h):
        E = E_tiles.pop(h)
        g, hg = divmod(h, GH)
        if hg == 0:
            cpq_new = sb.tile([128, NT, GH * U], BF16, tag="cpq", bufs=2)
            cp2_tiles[g] = cpq_new
        cpq = cp2_tiles[g]
        for nn in range(NT):
            c = ps_small.tile([128, VW], F32, tag="ps_c", bufs=2)
            for m in range(NT):
                nc.tensor.matmul(
                    c,
                    lhsT=E[:, m, nn * 128:(nn + 1) * 128],
                    rhs=VA[:, m, h * VW:(h + 1) * VW],
                    start=(m == 0), stop=(m == NT - 1),
                )
            r = sb.tile([128, 1], F32, tag="recip", bufs=8)
            nc.vector.reciprocal(out=r, in_=c[:, U:U + 1])
            nc.vector.tensor_scalar(out=cpq[:, nn, hg * U:(hg + 1) * U],
                                    in0=c[:, 0:U], scalar1=r,
                                    scalar2=None, op0=OP.mult)
            if hg == GH - 1:
                nc.sync.dma_start(
                    out=CT[:, g * TPG:(g + 1) * TPG, nn * 128:(nn + 1) * 128],
                    in_=cpq[:, nn, :], transpose=True)
        # fold the Wo accumulation once this group's CT tiles are complete
        if hg == GH - 1:
            first, last = (g == 0), (g == H // GH - 1)
            for q in range(2):
                if first:
                    wp = ps_big.tile([Uout, 512], F32, tag="ps_p", bufs=2)
                    wo_psum[q] = wp
                for ki in range(TPG):
                    k = g * TPG + ki
                    rhs = CT[:, k, q * 512:(q + 1) * 512]
                    nc.tensor.matmul(wo_psum[q], lhsT=Wo_hi[:, k, :], rhs=rhs,
                                     start=(first and ki == 0), stop=False)
                    nc.tensor.matmul(wo_psum[q], lhsT=Wo_lo[:, k, :], rhs=rhs,
                                     start=False, stop=(last and ki == TPG - 1))
            if last:
                for q in range(2):
                    nc.scalar.copy(out=HT_out[:, q * 512:(q + 1) * 512],
                                   in_=wo_psum[q])

    # head-0's QK projection chunk first so its S/exp stream starts early;
    # the remaining projections overlap the first heads' attention
    emitted_qk = set()

    def qk_if_needed(h, **kw):
        d = h // UD
        if h < H and d not in emitted_qk:
            emitted_qk.add(d)
            qk_proj(d, **kw)

    qk_if_needed(0)
    qk_if_needed(1)
    v_proj()
    qk_if_needed(2, tag="ps_c", pool=ps_small)
    s_phase(0)
    s_phase(1)
    for h in range(H):
        if h + 2 < H:
            qk_if_needed(h + 3, tag="ps_c", pool=ps_small)
            s_phase(h + 2)
        ctx_phase(h)


def build_nc(repeats=1):
    nc = bacc.Bacc("TRN2", target_bir_lowering=False, debug=False)

    x_d = nc.dram_tensor("X", [N, F], F32, kind="ExternalInput")
    a_d = nc.dram_tensor("A", [N, N], F32, kind="ExternalInput")
    w_d = {}
    shapes = {
        "Wq1": [F, H * U1], "Wk1": [F, H * U1], "Wv1": [F, H * U1],
        "Wo1": [H * U1, U1],
        "Wq2": [U1, H * U2], "Wk2": [U1, H * U2], "Wv2": [U1, H * U2],
        "Wo2": [H * U2, U2],
        "W1": [F, 32], "b1": [32], "W2": [32, 16], "b2": [16],
        "W3": [16, 1], "b3": [1],
    }
    for k, s in shapes.items():
        w_d[k] = nc.dram_tensor(k, s, F32, kind="ExternalInput")
    y_d = nc.dram_tensor("y", [1, 1], F32, kind="ExternalOutput")

    with tile.TileContext(nc) as tc:
        with (
            tc.tile_pool(name="sb", bufs=1) as sb,
            tc.tile_pool(name="scratch", bufs=2) as scratch,
            tc.tile_pool(name="ps_big", bufs=2, space="PSUM") as ps_big,
            tc.tile_pool(name="ps_small", bufs=2, space="PSUM") as ps_small,
        ):
          for _rep in range(repeats):
            ident = sb.tile([128, 128], BF16, tag="ident", bufs=1)
            make_identity(nc, ident)

            wq1, _ = _load_weight2d(nc, sb, scratch, "Wq1", w_d["Wq1"], F, H * U1, False)
            wk1, _ = _load_weight2d(nc, sb, scratch, "Wk1", w_d["Wk1"], F, H * U1, False)
            # ---- XT [F, N] bf16 via PE transpose ----
            x32 = sb.tile([128, NT, F], F32, tag="x32", bufs=1)
            nc.sync.dma_start(out=x32, in_=x_d[:].rearrange("(t p) f -> p t f", p=128))
            xb = sb.tile([128, NT, F], BF16, tag="xb", bufs=1)
            nc.vector.tensor_copy(out=xb, in_=x32)
            XT = sb.tile([F, N], BF16, tag="XT", bufs=1)
            for t in range(NT):
                pt = ps_small.tile([F, 128], BF16, tag="ps_c", bufs=2)
                nc.tensor.transpose(pt, xb[:, t, :], ident)
                nc.scalar.copy(out=XT[:, t * 128:(t + 1) * 128], in_=pt)

            # ---- A mask prep (DMA-heavy; scheduled before bulk weights) ----
            mn = sb.tile([128, NT, N], BF16, tag="E", bufs=4)
            for cc in range(NT):
                a32c = scratch.tile([128, N], F32, tag="a32c", bufs=2)
                nc.sync.dma_start(
                    out=a32c,
                    in_=a_d[:].rearrange("(t p) n -> p t n", p=128)[:, cc, :])
                nc.vector.tensor_scalar(out=mn[:, cc, :], in0=a32c, scalar1=-1.0,
                                        scalar2=1.0, op0=OP.mult, op1=OP.add)
            wv1h, wv1l = _load_weight2d(nc, sb, scratch, "Wv1", w_d["Wv1"], F, H * U1, True)
            wo1h, wo1l = _load_weight_kt(nc, sb, scratch, "Wo1", w_d["Wo1"], H * U1 // 128, U1, True)
            Mt = sb.tile([128, NT, N], BF16, tag="Mt", bufs=1)
            for cc in range(NT):
                nc.sync.dma_start(out=Mt[:, :, cc * 128:(cc + 1) * 128],
                                  in_=mn[:, cc, :], transpose=True)

            # ---- weights ----
            wq2, _ = _load_weight2d(nc, sb, scratch, "Wq2", w_d["Wq2"], U1, H * U2, False)
            wk2, _ = _load_weight2d(nc, sb, scratch, "Wk2", w_d["Wk2"], U1, H * U2, False)
            wv2h, wv2l = _load_weight2d(nc, sb, scratch, "Wv2", w_d["Wv2"], U1, H * U2, True)
            wo2h, wo2l = _load_weight_kt(nc, sb, scratch, "Wo2", w_d["Wo2"], H * U2 // 128, U2, True)

            w1 = sb.tile([F, 32], F32, tag="W1", bufs=1)
            nc.sync.dma_start(out=w1, in_=w_d["W1"][:])
            w2 = sb.tile([32, 16], F32, tag="W2", bufs=1)
            nc.sync.dma_start(out=w2, in_=w_d["W2"][:])
            w3 = sb.tile([16, 1], F32, tag="W3", bufs=1)
            nc.sync.dma_start(out=w3, in_=w_d["W3"][:])
            b1 = sb.tile([32, 1], F32, tag="b1", bufs=1)
            nc.sync.dma_start(out=b1, in_=w_d["b1"][:].rearrange("(p x) -> p x", x=1))
            b2 = sb.tile([16, 1], F32, tag="b2", bufs=1)
            nc.sync.dma_start(out=b2, in_=w_d["b2"][:].rearrange("(p x) -> p x", x=1))
            b3 = sb.tile([1, 1], F32, tag="b3", bufs=1)
            nc.sync.dma_start(out=b3, in_=w_d["b3"][:].rearrange("(p x) -> p x", x=1))

            # ---- layers ----
            H1T = sb.tile([U1, N], BF16, tag="H1T", bufs=1)
            _gat_layer(nc, sb, ps_big, ps_small, XT, U1, U1,
                       wq1, wk1, wv1h, wv1l, wo1h, wo1l, Mt, H1T)
            H2T = sb.tile([U2, N], BF16, tag="H2T", bufs=1)
            _gat_layer(nc, sb, ps_big, ps_small, H1T, U2, U2,
                       wq2, wk2, wv2h, wv2l, wo2h, wo2l, Mt, H2T)

            # ---- mean pool + MLP ----
            hs2 = sb.tile([U2, 2], F32, tag="hsum2", bufs=1)
            for q in range(2):
                nc.vector.reduce_sum(out=hs2[:, q:q + 1],
                                     in_=H2T[:, q * 512:(q + 1) * 512],
                                     axis=mybir.AxisListType.X)
            hs = sb.tile([U2, 1], F32, tag="hsum", bufs=1)
            nc.vector.tensor_add(out=hs, in0=hs2[:, 0:1], in1=hs2[:, 1:2])
            p1 = ps_small.tile([32, 1], F32, tag="ps_c", bufs=2)
            nc.tensor.matmul(p1, lhsT=w1, rhs=hs, start=True, stop=True)
            a1 = sb.tile([32, 1], F32, tag="a1", bufs=1)
            nc.scalar.activation(out=a1, in_=p1, func=AF.Relu, bias=b1,
                                 scale=1.0 / float(N))
            p2 = ps_small.tile([16, 1], F32, tag="ps_c", bufs=2)
            nc.tensor.matmul(p2, lhsT=w2, rhs=a1, start=True, stop=True)
            a2 = sb.tile([16, 1], F32, tag="a2", bufs=1)
            nc.scalar.activation(out=a2, in_=p2, func=AF.Relu, bias=b2)
            p3 = ps_small.tile([1, 1], F32, tag="ps_c", bufs=2)
            nc.tensor.matmul(p3, lhsT=w3, rhs=a2, start=True, stop=True)
            yt = sb.tile([1, 1], F32, tag="yt", bufs=1)
            nc.vector.tensor_add(out=yt, in0=p3, in1=b3)
            nc.sync.dma_start(out=y_d[:], in_=yt)

    nc.compile()
    return nc


_NC = None


def _get_nc():
    global _NC
    if _NC is None:
        _NC = build_nc()
    return _NC


def make_in_maps(inputs):
    in_maps = []
    for i in range(B):
        m = {"X": np.ascontiguousarray(np.asarray(inputs["X"][i], dtype=np.float32)),
             "A": np.ascontiguousarray(np.asarray(inputs["A"][i], dtype=np.float32))}
        for k in WEIGHT_NAMES:
            m[k] = np.ascontiguousarray(np.asarray(inputs[k], dtype=np.float32))
        in_maps.append(m)
    return in_maps


def run(inputs, trace=False):
    nc = _get_nc()
    res = run_bass_kernel_spmd(nc, make_in_maps(inputs), list(range(B)), trace=trace)
    y = np.stack([res.results[i]["y"][0] for i in range(B)], axis=0)
    return y.astype(np.float32), res


def kernel(**inputs):
    y, _ = run(inputs, trace=False)
    return y



# revision 23
# speedup vs baseline: 1.3276x; 1.0376x over previous
"""Trainium2 Bass kernel for nn_CustomGNN_66881230733874 (2-layer GAT + mean-pool + MLP).

Sharding: data-parallel over batch B=8 -> one graph per NeuronCore (8 cores).
Each core computes its full graph end-to-end (no collectives); host gathers [8,1].

Layout strategy (per core):
  - Activations live feature-on-partition ("transposed"): XT [F, N], QT/KT [(h,d), N].
  - Scores computed directly transposed: S^T[m, n] = K_h Q_h^T via PE.
  - softmax without max-subtraction (scores are O(1) by construction);
    masked entries are exactly zeroed by multiplying exp(S) with (1-A)^T.
  - ctx_unnorm [n, u] via lhsT=E[m,n-chunk], rhs=V_aug[m, u+1] where V_aug has a
    ones column -> last PSUM column accumulates the softmax denominator Z[n].
  - normalize with per-partition reciprocal (DVE), pack bf16, DMA-transpose
    (SBUF xbar) to get ctx^T [(h,u), N] for the W_out matmul. No PE transposes.
  - Weight matmuls use split-bf16 (hi+lo) for the value path (Wv, Wo) to kill
    correlated rounding error; Wq/Wk single-bf16 (score path is tolerant).
"""

import numpy as np

import concourse.bass as bass
import concourse.mybir as mybir
import concourse.tile as tile
from concourse import bacc
from concourse.bass_utils import run_bass_kernel_spmd
from concourse.masks import make_identity

F32 = mybir.dt.float32
BF16 = mybir.dt.bfloat16
AF = mybir.ActivationFunctionType
OP = mybir.AluOpType

B = 8
N = 1024
F = 64
H = 8
U1, U2 = 128, 64
NT = N // 128  # 8 node chunks

WEIGHT_NAMES = [
    "Wq1", "Wk1", "Wv1", "Wo1", "Wq2", "Wk2", "Wv2", "Wo2",
    "W1", "b1", "W2", "b2", "W3", "b3",
]


def _load_weight2d(nc, sb, scratch, name, dram, part, cols, split):
    """DMA a [part, cols] f32 weight to SBUF bf16 (hi[, lo])."""
    w32 = scratch.tile([part, cols], F32, tag="wscratch", bufs=2)
    nc.sync.dma_start(out=w32, in_=dram[:])
    hi = sb.tile([part, cols], BF16, tag=f"w_{name}_hi", bufs=1)
    nc.vector.tensor_copy(out=hi, in_=w32)
    lo = None
    if split:
        lo = sb.tile([part, cols], BF16, tag=f"w_{name}_lo", bufs=1)
        nc.vector.tensor_sub(out=lo, in0=w32, in1=hi)
    return hi, lo


def _load_weight_kt(nc, sb, scratch, name, dram, kt, cols, split):
    """DMA a [kt*128, cols] f32 weight to SBUF bf16 [128, kt, cols] (hi, lo)."""
    w32 = scratch.tile([128, kt, cols], F32, tag="wscratch", bufs=2)
    nc.sync.dma_start(out=w32, in_=dram[:].rearrange("(k p) c -> p k c", p=128))
    hi = sb.tile([128, kt, cols], BF16, tag=f"w_{name}_hi", bufs=1)
    nc.vector.tensor_copy(out=hi, in_=w32)
    lo = sb.tile([128, kt, cols], BF16, tag=f"w_{name}_lo", bufs=1)
    nc.vector.tensor_sub(out=lo, in0=w32, in1=hi)
    return hi, lo


def _gat_layer(nc, sb, ps_big, ps_small, XT, U, Uout,
               Wq, Wk, Wv_hi, Wv_lo, Wo_hi, Wo_lo, Mt, HT_out):
    """One GAT layer. XT: bf16 [Cin, N]. Writes HT_out: bf16 [Uout, N]."""
    HU = H * U
    DT = HU // 128          # number of 128-row tiles of QT/KT
    UD = 128 // U           # heads per QT/KT tile
    VW = U + 1              # V_aug width per head
    inv_sqrt_u = 1.0 / float(np.sqrt(U))

    # ---- projections: QT/KT [(h,d), N] bf16; V_aug [n, H*(U+1)] bf16 ----
    QT = sb.tile([128, DT, N], BF16, tag="QT", bufs=1)
    KT = sb.tile([128, DT, N], BF16, tag="KT", bufs=1)
    VA = sb.tile([128, NT, H * VW], BF16, tag="VA", bufs=1)

    def qk_proj(d, tag="ps_p", pool=None):
        for w, dst, on_act in ((Wq, QT, True), (Wk, KT, False)):
            for q in range(2):
                p = (pool or ps_big).tile([128, 512], F32, tag=tag, bufs=2)
                nc.tensor.matmul(p, lhsT=w[:, d * 128:(d + 1) * 128],
                                 rhs=XT[:, q * 512:(q + 1) * 512],
                                 start=True, stop=True)
                if on_act:
                    nc.scalar.copy(out=dst[:, d, q * 512:(q + 1) * 512], in_=p)
                else:
                    nc.vector.tensor_copy(out=dst[:, d, q * 512:(q + 1) * 512],
                                          in_=p)

    def v_proj():
        n_vc = HU // 512  # 512-wide chunks of H*U
        hpc = 512 // U    # heads per 512 chunk
        for m in range(NT):
            VAm = VA[:, m, :].rearrange("p (h x) -> p h x", h=H)
            nc.vector.memset(VAm[:, :, U:U + 1], 1.0)
        for c in range(n_vc):
            for m in range(NT):
                VAm = VA[:, m, :].rearrange("p (h x) -> p h x", h=H)
                p = ps_big.tile([128, 512], F32, tag="ps_p", bufs=2)
                xm = XT[:, m * 128:(m + 1) * 128]
                nc.tensor.matmul(p, lhsT=xm, rhs=Wv_hi[:, c * 512:(c + 1) * 512],
                                 start=True, stop=False)
                nc.tensor.matmul(p, lhsT=xm, rhs=Wv_lo[:, c * 512:(c + 1) * 512],
                                 start=False, stop=True)
                nc.vector.tensor_copy(
                    out=VAm[:, c * hpc:(c + 1) * hpc, 0:U],
                    in_=p.rearrange("p (h x) -> p h x", h=hpc),
                )

    # ---- attention, software-pipelined: S(h+1) emitted before ctx(h) so the
    # exp stream on ACT is never starved by PE's ctx block. m-chunks are
    # processed in pairs (FD=2048) to halve per-instruction overheads ----
    CT = sb.tile([128, DT, N], BF16, tag="CT", bufs=1)  # ctx^T [(h,u), N]
    E_tiles = {}
    cp2_tiles = {}
    wo_psum = [None, None]
    KD = HU // 128

    def s_phase(h):
        kt_i, k_off = h // UD, (h % UD) * U
        E = sb.tile([128, NT, N], BF16, tag="E", bufs=4)
        E_tiles[h] = E
        for m in range(NT):
            s = ps_big.tile([128, 1024], F32, tag="ps_s", bufs=2)
            for q in range(2):
                nc.tensor.matmul(
                    s[:, q * 512:(q + 1) * 512],
                    lhsT=KT[k_off:k_off + U, kt_i, m * 128:(m + 1) * 128],
                    rhs=QT[k_off:k_off + U, kt_i, q * 512:(q + 1) * 512],
                    start=True, stop=True,
                )
            nc.scalar.activation(out=E[:, m, :], in_=s, func=AF.Exp,
                                 scale=inv_sqrt_u)
            nc.vector.tensor_mul(out=E[:, m, :], in0=E[:, m, :], in1=Mt[:, m, :])

    GH = 2                    # heads per transpose/Wo group
    TPG = GH * U // 128       # CT tiles per group

    def ctx_phase(h):
        E = E_tiles.pop(h)
        g, hg = divmod(h, GH)
        batched_t = (U == 128)  # head-major cpq -> one xbar transpose per group
        if hg == 0:
            if batched_t:
                cpq_new = sb.tile([128, GH, NT, U], BF16, tag="cpq", bufs=2)
            else:
                cpq_new = sb.tile([128, NT, GH * U], BF16, tag="cpq", bufs=2)
            cp2_tiles[g] = cpq_new
        cpq = cp2_tiles[g]
        for nn in range(NT):
            c = ps_small.tile([128, VW], F32, tag="ps_c", bufs=2)
            for m in range(NT):
                nc.tensor.matmul(
                    c,
                    lhsT=E[:, m, nn * 128:(nn + 1) * 128],
                    rhs=VA[:, m, h * VW:(h + 1) * VW],
                    start=(m == 0), stop=(m == NT - 1),
                )
            r = sb.tile([128, 1], F32, tag="recip", bufs=8)
            nc.vector.reciprocal(out=r, in_=c[:, U:U + 1])
            dst = (cpq[:, hg, nn, :] if batched_t
                   else cpq[:, nn, hg * U:(hg + 1) * U])
            nc.vector.tensor_scalar(out=dst, in0=c[:, 0:U], scalar1=r,
                                    scalar2=None, op0=OP.mult)
            if not batched_t and hg == GH - 1:
                nc.sync.dma_start(
                    out=CT[:, g * TPG:(g + 1) * TPG, nn * 128:(nn + 1) * 128],
                    in_=cpq[:, nn, :], transpose=True)
        # batched layer-1 transpose: cpq free index c = hg*NT*U + nn*U + u
        # lands at out[u, hg*NT+nn, p] = CT[u, g*GH+hg, nn*128+p]
        if batched_t and hg == GH - 1:
            nc.sync.dma_start(
                out=CT[:, g * TPG:(g + 1) * TPG, :].rearrange(
                    "p k (t c) -> p k t c", c=128),
                in_=cpq[:, :, :, :], transpose=True)
        # fold the Wo accumulation once this group's CT tiles are complete
        if hg == GH - 1:
            first, last = (g == 0), (g == H // GH - 1)
            for q in range(2):
                if first:
                    wp = ps_big.tile([Uout, 512], F32, tag="ps_p", bufs=2)
                    wo_psum[q] = wp
                for ki in range(TPG):
                    k = g * TPG + ki
                    rhs = CT[:, k, q * 512:(q + 1) * 512]
                    nc.tensor.matmul(wo_psum[q], lhsT=Wo_hi[:, k, :], rhs=rhs,
                                     start=(first and ki == 0), stop=False)
                    nc.tensor.matmul(wo_psum[q], lhsT=Wo_lo[:, k, :], rhs=rhs,
                                     start=False, stop=(last and ki == TPG - 1))
            if last:
                for q in range(2):
                    nc.scalar.copy(out=HT_out[:, q * 512:(q + 1) * 512],
                                   in_=wo_psum[q])

    # head-0's QK projection chunk first so its S/exp stream starts early;
    # the remaining projections overlap the first heads' attention
    emitted_qk = set()

    def qk_if_needed(h, **kw):
        d = h // UD
        if h < H and d not in emitted_qk:
            emitted_qk.add(d)
            qk_proj(d, **kw)

    qk_if_needed(0)
    qk_if_needed(1)
    v_proj()
    qk_if_needed(2, tag="ps_c", pool=ps_small)
    s_phase(0)
    s_phase(1)
    for h in range(H):
        if h + 2 < H:
            qk_if_needed(h + 3, tag="ps_c", pool=ps_small)
            s_phase(h + 2)
        ctx_phase(h)


def build_nc(repeats=1):
    nc = bacc.Bacc("TRN2", target_bir_lowering=False, debug=False)

    x_d = nc.dram_tensor("X", [N, F], F32, kind="ExternalInput")
    a_d = nc.dram_tensor("A", [N, N], F32, kind="ExternalInput")
    w_d = {}
    shapes = {
        "Wq1": [F, H * U1], "Wk1": [F, H * U1], "Wv1": [F, H * U1],
        "Wo1": [H * U1, U1],
        "Wq2": [U1, H * U2], "Wk2": [U1, H * U2], "Wv2": [U1, H * U2],
        "Wo2": [H * U2, U2],
        "W1": [F, 32], "b1": [32], "W2": [32, 16], "b2": [16],
        "W3": [16, 1], "b3": [1],
    }
    for k, s in shapes.items():
        w_d[k] = nc.dram_tensor(k, s, F32, kind="ExternalInput")
    y_d = nc.dram_tensor("y", [1, 1], F32, kind="ExternalOutput")

    with tile.TileContext(nc) as tc:
        with (
            tc.tile_pool(name="sb", bufs=1) as sb,
            tc.tile_pool(name="scratch", bufs=2) as scratch,
            tc.tile_pool(name="ps_big", bufs=2, space="PSUM") as ps_big,
            tc.tile_pool(name="ps_small", bufs=2, space="PSUM") as ps_small,
        ):
          for _rep in range(repeats):
            ident = sb.tile([128, 128], BF16, tag="ident", bufs=1)
            make_identity(nc, ident)

            wq1, _ = _load_weight2d(nc, sb, scratch, "Wq1", w_d["Wq1"], F, H * U1, False)
            wk1, _ = _load_weight2d(nc, sb, scratch, "Wk1", w_d["Wk1"], F, H * U1, False)
            # ---- XT [F, N] bf16 via PE transpose ----
            x32 = sb.tile([128, NT, F], F32, tag="x32", bufs=1)
            nc.sync.dma_start(out=x32, in_=x_d[:].rearrange("(t p) f -> p t f", p=128))
            xb = sb.tile([128, NT, F], BF16, tag="xb", bufs=1)
            nc.vector.tensor_copy(out=xb, in_=x32)
            XT = sb.tile([F, N], BF16, tag="XT", bufs=1)
            for t in range(NT):
                pt = ps_small.tile([F, 128], BF16, tag="ps_c", bufs=2)
                nc.tensor.transpose(pt, xb[:, t, :], ident)
                nc.scalar.copy(out=XT[:, t * 128:(t + 1) * 128], in_=pt)

            # ---- A mask prep (DMA-heavy; scheduled before bulk weights) ----
            mn = sb.tile([128, NT, N], BF16, tag="E", bufs=4)
            for cc in range(NT):
                a32c = scratch.tile([128, N], F32, tag="a32c", bufs=2)
                nc.sync.dma_start(
                    out=a32c,
                    in_=a_d[:].rearrange("(t p) n -> p t n", p=128)[:, cc, :])
                nc.vector.tensor_scalar(out=mn[:, cc, :], in0=a32c, scalar1=-1.0,
                                        scalar2=1.0, op0=OP.mult, op1=OP.add)
            wv1h, wv1l = _load_weight2d(nc, sb, scratch, "Wv1", w_d["Wv1"], F, H * U1, True)
            wo1h, wo1l = _load_weight_kt(nc, sb, scratch, "Wo1", w_d["Wo1"], H * U1 // 128, U1, True)
            Mt = sb.tile([128, NT, N], BF16, tag="Mt", bufs=1)
            for cc in range(NT):
                nc.sync.dma_start(out=Mt[:, :, cc * 128:(cc + 1) * 128],
                                  in_=mn[:, cc, :], transpose=True)

            # ---- weights ----
            wq2, _ = _load_weight2d(nc, sb, scratch, "Wq2", w_d["Wq2"], U1, H * U2, False)
            wk2, _ = _load_weight2d(nc, sb, scratch, "Wk2", w_d["Wk2"], U1, H * U2, False)
            wv2h, wv2l = _load_weight2d(nc, sb, scratch, "Wv2", w_d["Wv2"], U1, H * U2, True)
            wo2h, wo2l = _load_weight_kt(nc, sb, scratch, "Wo2", w_d["Wo2"], H * U2 // 128, U2, True)

            w1 = sb.tile([F, 32], F32, tag="W1", bufs=1)
            nc.sync.dma_start(out=w1, in_=w_d["W1"][:])
            w2 = sb.tile([32, 16], F32, tag="W2", bufs=1)
            nc.sync.dma_start(out=w2, in_=w_d["W2"][:])
            w3 = sb.tile([16, 1], F32, tag="W3", bufs=1)
            nc.sync.dma_start(out=w3, in_=w_d["W3"][:])
            b1 = sb.tile([32, 1], F32, tag="b1", bufs=1)
            nc.sync.dma_start(out=b1, in_=w_d["b1"][:].rearrange("(p x) -> p x", x=1))
            b2 = sb.tile([16, 1], F32, tag="b2", bufs=1)
            nc.sync.dma_start(out=b2, in_=w_d["b2"][:].rearrange("(p x) -> p x", x=1))
            b3 = sb.tile([1, 1], F32, tag="b3", bufs=1)
            nc.sync.dma_start(out=b3, in_=w_d["b3"][:].rearrange("(p x) -> p x", x=1))

            # ---- layers ----
            H1T = sb.tile([U1, N], BF16, tag="H1T", bufs=1)
            _gat_layer(nc, sb, ps_big, ps_small, XT, U1, U1,
                       wq1, wk1, wv1h, wv1l, wo1h, wo1l, Mt, H1T)
            H2T = sb.tile([U2, N], BF16, tag="H2T", bufs=1)
            _gat_layer(nc, sb, ps_big, ps_small, H1T, U2, U2,
                       wq2, wk2, wv2h, wv2l, wo2h, wo2l, Mt, H2T)

            # ---- mean pool + MLP ----
            hs2 = sb.tile([U2, 2], F32, tag="hsum2", bufs=1)
            for q in range(2):
                nc.vector.reduce_sum(out=hs2[:, q:q + 1],
                                     in_=H2T[:, q * 512:(q + 1) * 512],
                                     axis=mybir.AxisListType.X)
            hs = sb.tile([U2, 1], F32, tag="hsum", bufs=1)
            nc.vector.tensor_add(out=hs, in0=hs2[:, 0:1], in1=hs2[:, 1:2])
            p1 = ps_small.tile([32, 1], F32, tag="ps_c", bufs=2)
            nc.tensor.matmul(p1, lhsT=w1, rhs=hs, start=True, stop=True)
            a1 = sb.tile([32, 1], F32, tag="a1", bufs=1)
            nc.scalar.activation(out=a1, in_=p1, func=AF.Relu, bias=b1,
                                 scale=1.0 / float(N))
            p2 = ps_small.tile([16, 1], F32, tag="ps_c", bufs=2)
            nc.tensor.matmul(p2, lhsT=w2, rhs=a1, start=True, stop=True)
            a2 = sb.tile([16, 1], F32, tag="a2", bufs=1)
            nc.scalar.activation(out=a2, in_=p2, func=AF.Relu, bias=b2)
            p3 = ps_small.tile([1, 1], F32, tag="ps_c", bufs=2)
            nc.tensor.matmul(p3, lhsT=w3, rhs=a2, start=True, stop=True)
            yt = sb.tile([1, 1], F32, tag="yt", bufs=1)
            nc.vector.tensor_add(out=yt, in0=p3, in1=b3)
            nc.sync.dma_start(out=y_d[:], in_=yt)

    nc.compile()
    return nc


_NC = None


def _get_nc():
    global _NC
    if _NC is None:
        _NC = build_nc()
    return _NC


def make_in_maps(inputs):
    in_maps = []
    for i in range(B):
        m = {"X": np.ascontiguousarray(np.asarray(inputs["X"][i], dtype=np.float32)),
             "A": np.ascontiguousarray(np.asarray(inputs["A"][i], dtype=np.float32))}
        for k in WEIGHT_NAMES:
            m[k] = np.ascontiguousarray(np.asarray(inputs[k], dtype=np.float32))
        in_maps.append(m)
    return in_maps


def run(inputs, trace=False):
    nc = _get_nc()
    res = run_bass_kernel_spmd(nc, make_in_maps(inputs), list(range(B)), trace=trace)
    y = np.stack([res.results[i]["y"][0] for i in range(B)], axis=0)
    return y.astype(np.float32), res


def kernel(**inputs):
    y, _ = run(inputs, trace=False)
    return y



# revision 24
# speedup vs baseline: 1.3747x; 1.0355x over previous
"""Trainium2 Bass kernel for nn_CustomGNN_66881230733874 (2-layer GAT + mean-pool + MLP).

Sharding: data-parallel over batch B=8 -> one graph per NeuronCore (8 cores).
Each core computes its full graph end-to-end (no collectives); host gathers [8,1].

Layout strategy (per core):
  - Activations live feature-on-partition ("transposed"): XT [F, N], QT/KT [(h,d), N].
  - Scores computed directly transposed: S^T[m, n] = K_h Q_h^T via PE.
  - softmax without max-subtraction (scores are O(1) by construction);
    masked entries are exactly zeroed by multiplying exp(S) with (1-A)^T.
  - ctx_unnorm [n, u] via lhsT=E[m,n-chunk], rhs=V_aug[m, u+1] where V_aug has a
    ones column -> last PSUM column accumulates the softmax denominator Z[n].
  - normalize with per-partition reciprocal (DVE), pack bf16, DMA-transpose
    (SBUF xbar) to get ctx^T [(h,u), N] for the W_out matmul. No PE transposes.
  - Weight matmuls use split-bf16 (hi+lo) for the value path (Wv, Wo) to kill
    correlated rounding error; Wq/Wk single-bf16 (score path is tolerant).
"""

import numpy as np

import concourse.bass as bass
import concourse.mybir as mybir
import concourse.tile as tile
from concourse import bacc
from concourse.bass_utils import run_bass_kernel_spmd
from concourse.masks import make_identity

F32 = mybir.dt.float32
BF16 = mybir.dt.bfloat16
AF = mybir.ActivationFunctionType
OP = mybir.AluOpType

B = 8
N = 1024
F = 64
H = 8
U1, U2 = 128, 64
NT = N // 128  # 8 node chunks

WEIGHT_NAMES = [
    "Wq1", "Wk1", "Wv1", "Wo1", "Wq2", "Wk2", "Wv2", "Wo2",
    "W1", "b1", "W2", "b2", "W3", "b3",
]


def _load_weight2d(nc, sb, scratch, name, dram, part, cols, split):
    """DMA a [part, cols] f32 weight to SBUF bf16 (hi[, lo])."""
    w32 = scratch.tile([part, cols], F32, tag="wscratch", bufs=2)
    nc.sync.dma_start(out=w32, in_=dram[:])
    hi = sb.tile([part, cols], BF16, tag=f"w_{name}_hi", bufs=1)
    nc.vector.tensor_copy(out=hi, in_=w32)
    lo = None
    if split:
        lo = sb.tile([part, cols], BF16, tag=f"w_{name}_lo", bufs=1)
        nc.vector.tensor_sub(out=lo, in0=w32, in1=hi)
    return hi, lo


def _load_weight_kt(nc, sb, scratch, name, dram, kt, cols, split):
    """DMA a [kt*128, cols] f32 weight to SBUF bf16 [128, kt, cols] (hi, lo)."""
    w32 = scratch.tile([128, kt, cols], F32, tag="wscratch", bufs=2)
    nc.sync.dma_start(out=w32, in_=dram[:].rearrange("(k p) c -> p k c", p=128))
    hi = sb.tile([128, kt, cols], BF16, tag=f"w_{name}_hi", bufs=1)
    nc.vector.tensor_copy(out=hi, in_=w32)
    lo = sb.tile([128, kt, cols], BF16, tag=f"w_{name}_lo", bufs=1)
    nc.vector.tensor_sub(out=lo, in0=w32, in1=hi)
    return hi, lo


def _gat_layer(nc, sb, ps_big, ps_small, XT, U, Uout,
               Wq, Wk, Wv_hi, Wv_lo, Wo_hi, Wo_lo, Mt, HT_out):
    """One GAT layer. XT: bf16 [Cin, N]. Writes HT_out: bf16 [Uout, N]."""
    HU = H * U
    DT = HU // 128          # number of 128-row tiles of QT/KT
    UD = 128 // U           # heads per QT/KT tile
    VW = U + 1              # V_aug width per head
    inv_sqrt_u = 1.0 / float(np.sqrt(U))

    # ---- projections: QT/KT [(h,d), N] bf16; V_aug [n, H*(U+1)] bf16 ----
    QT = sb.tile([128, DT, N], BF16, tag="QT", bufs=1)
    KT = sb.tile([128, DT, N], BF16, tag="KT", bufs=1)
    VA = sb.tile([128, NT, H * VW], BF16, tag="VA", bufs=1)

    def qk_proj(d, tag="ps_p", pool=None):
        for w, dst, on_act in ((Wq, QT, True), (Wk, KT, False)):
            for q in range(2):
                p = (pool or ps_big).tile([128, 512], F32, tag=tag, bufs=2)
                nc.tensor.matmul(p, lhsT=w[:, d * 128:(d + 1) * 128],
                                 rhs=XT[:, q * 512:(q + 1) * 512],
                                 start=True, stop=True)
                if on_act:
                    nc.scalar.copy(out=dst[:, d, q * 512:(q + 1) * 512], in_=p)
                else:
                    nc.vector.tensor_copy(out=dst[:, d, q * 512:(q + 1) * 512],
                                          in_=p)

    def v_proj():
        n_vc = HU // 512  # 512-wide chunks of H*U
        hpc = 512 // U    # heads per 512 chunk
        for m in range(NT):
            VAm = VA[:, m, :].rearrange("p (h x) -> p h x", h=H)
            nc.vector.memset(VAm[:, :, U:U + 1], 1.0)
        for c in range(n_vc):
            for m in range(NT):
                VAm = VA[:, m, :].rearrange("p (h x) -> p h x", h=H)
                p = ps_big.tile([128, 512], F32, tag="ps_p", bufs=2)
                xm = XT[:, m * 128:(m + 1) * 128]
                nc.tensor.matmul(p, lhsT=xm, rhs=Wv_hi[:, c * 512:(c + 1) * 512],
                                 start=True, stop=False)
                nc.tensor.matmul(p, lhsT=xm, rhs=Wv_lo[:, c * 512:(c + 1) * 512],
                                 start=False, stop=True)
                nc.vector.tensor_copy(
                    out=VAm[:, c * hpc:(c + 1) * hpc, 0:U],
                    in_=p.rearrange("p (h x) -> p h x", h=hpc),
                )

    # ---- attention, software-pipelined: S(h+1) emitted before ctx(h) so the
    # exp stream on ACT is never starved by PE's ctx block. m-chunks are
    # processed in pairs (FD=2048) to halve per-instruction overheads ----
    CT = sb.tile([128, DT, N], BF16, tag="CT", bufs=1)  # ctx^T [(h,u), N]
    E_tiles = {}
    cp2_tiles = {}
    wo_psum = [None, None]
    KD = HU // 128

    def s_phase(h):
        kt_i, k_off = h // UD, (h % UD) * U
        E = sb.tile([128, NT, N], BF16, tag="E", bufs=4)
        E_tiles[h] = E
        for m in range(NT):
            s = ps_big.tile([128, 1024], F32, tag="ps_s", bufs=2)
            for q in range(2):
                nc.tensor.matmul(
                    s[:, q * 512:(q + 1) * 512],
                    lhsT=KT[k_off:k_off + U, kt_i, m * 128:(m + 1) * 128],
                    rhs=QT[k_off:k_off + U, kt_i, q * 512:(q + 1) * 512],
                    start=True, stop=True,
                )
            nc.scalar.activation(out=E[:, m, :], in_=s, func=AF.Exp,
                                 scale=inv_sqrt_u)
            nc.vector.tensor_mul(out=E[:, m, :], in0=E[:, m, :], in1=Mt[:, m, :])

    GH = 2                    # heads per transpose/Wo group
    TPG = GH * U // 128       # CT tiles per group

    def ctx_phase(h):
        E = E_tiles.pop(h)
        g, hg = divmod(h, GH)
        batched_t = (U == 128)  # head-major cpq -> one xbar transpose per group
        if hg == 0:
            if batched_t:
                cpq_new = sb.tile([128, GH, NT, U], BF16, tag="cpq", bufs=2)
            else:
                cpq_new = sb.tile([128, NT, GH * U], BF16, tag="cpq", bufs=2)
            cp2_tiles[g] = cpq_new
        cpq = cp2_tiles[g]
        for nn in range(NT):
            c = ps_small.tile([128, VW], F32, tag="ps_c", bufs=2)
            for m in range(NT):
                nc.tensor.matmul(
                    c,
                    lhsT=E[:, m, nn * 128:(nn + 1) * 128],
                    rhs=VA[:, m, h * VW:(h + 1) * VW],
                    start=(m == 0), stop=(m == NT - 1),
                )
            r = sb.tile([128, 1], F32, tag="recip", bufs=8)
            nc.vector.reciprocal(out=r, in_=c[:, U:U + 1])
            dst = (cpq[:, hg, nn, :] if batched_t
                   else cpq[:, nn, hg * U:(hg + 1) * U])
            nc.vector.tensor_scalar(out=dst, in0=c[:, 0:U], scalar1=r,
                                    scalar2=None, op0=OP.mult)
        # one xbar transpose per group: free index c lands at
        # out[c%128, c//128, p]; both layouts put (head-row, tile) in
        # (c%128 resp. c//128) so the same CT view works.
        if hg == GH - 1:
            nc.sync.dma_start(
                out=CT[:, g * TPG:(g + 1) * TPG, :].rearrange(
                    "p k (t c) -> p k t c", c=128),
                in_=(cpq[:, :, :, :] if batched_t else cpq[:, :, :]),
                transpose=True)
        # fold the Wo accumulation once this group's CT tiles are complete
        if hg == GH - 1:
            first, last = (g == 0), (g == H // GH - 1)
            for q in range(2):
                if first:
                    wp = ps_big.tile([Uout, 512], F32, tag="ps_p", bufs=2)
                    wo_psum[q] = wp
                for ki in range(TPG):
                    k = g * TPG + ki
                    rhs = CT[:, k, q * 512:(q + 1) * 512]
                    nc.tensor.matmul(wo_psum[q], lhsT=Wo_hi[:, k, :], rhs=rhs,
                                     start=(first and ki == 0), stop=False)
                    nc.tensor.matmul(wo_psum[q], lhsT=Wo_lo[:, k, :], rhs=rhs,
                                     start=False, stop=(last and ki == TPG - 1))
            if last:
                for q in range(2):
                    nc.scalar.copy(out=HT_out[:, q * 512:(q + 1) * 512],
                                   in_=wo_psum[q])

    # head-0's QK projection chunk first so its S/exp stream starts early;
    # the remaining projections overlap the first heads' attention
    emitted_qk = set()

    def qk_if_needed(h, **kw):
        d = h // UD
        if h < H and d not in emitted_qk:
            emitted_qk.add(d)
            qk_proj(d, **kw)

    qk_if_needed(0)
    qk_if_needed(1)
    v_proj()
    qk_if_needed(2, tag="ps_c", pool=ps_small)
    s_phase(0)
    s_phase(1)
    for h in range(H):
        if h + 2 < H:
            qk_if_needed(h + 3, tag="ps_c", pool=ps_small)
            s_phase(h + 2)
        ctx_phase(h)


def build_nc(repeats=1):
    nc = bacc.Bacc("TRN2", target_bir_lowering=False, debug=False)

    x_d = nc.dram_tensor("X", [N, F], F32, kind="ExternalInput")
    a_d = nc.dram_tensor("A", [N, N], F32, kind="ExternalInput")
    w_d = {}
    shapes = {
        "Wq1": [F, H * U1], "Wk1": [F, H * U1], "Wv1": [F, H * U1],
        "Wo1": [H * U1, U1],
        "Wq2": [U1, H * U2], "Wk2": [U1, H * U2], "Wv2": [U1, H * U2],
        "Wo2": [H * U2, U2],
        "W1": [F, 32], "b1": [32], "W2": [32, 16], "b2": [16],
        "W3": [16, 1], "b3": [1],
    }
    for k, s in shapes.items():
        w_d[k] = nc.dram_tensor(k, s, F32, kind="ExternalInput")
    y_d = nc.dram_tensor("y", [1, 1], F32, kind="ExternalOutput")

    with tile.TileContext(nc) as tc:
        with (
            tc.tile_pool(name="sb", bufs=1) as sb,
            tc.tile_pool(name="scratch", bufs=2) as scratch,
            tc.tile_pool(name="ps_big", bufs=2, space="PSUM") as ps_big,
            tc.tile_pool(name="ps_small", bufs=2, space="PSUM") as ps_small,
        ):
          for _rep in range(repeats):
            ident = sb.tile([128, 128], BF16, tag="ident", bufs=1)
            make_identity(nc, ident)

            wq1, _ = _load_weight2d(nc, sb, scratch, "Wq1", w_d["Wq1"], F, H * U1, False)
            wk1, _ = _load_weight2d(nc, sb, scratch, "Wk1", w_d["Wk1"], F, H * U1, False)
            # ---- XT [F, N] bf16 via PE transpose ----
            x32 = sb.tile([128, NT, F], F32, tag="x32", bufs=1)
            nc.sync.dma_start(out=x32, in_=x_d[:].rearrange("(t p) f -> p t f", p=128))
            xb = sb.tile([128, NT, F], BF16, tag="xb", bufs=1)
            nc.vector.tensor_copy(out=xb, in_=x32)
            XT = sb.tile([F, N], BF16, tag="XT", bufs=1)
            for t in range(NT):
                pt = ps_small.tile([F, 128], BF16, tag="ps_c", bufs=2)
                nc.tensor.transpose(pt, xb[:, t, :], ident)
                nc.scalar.copy(out=XT[:, t * 128:(t + 1) * 128], in_=pt)

            # ---- A mask prep (DMA-heavy; scheduled before bulk weights) ----
            mn = sb.tile([128, NT, N], BF16, tag="E", bufs=4)
            for cc in range(NT):
                a32c = scratch.tile([128, N], F32, tag="a32c", bufs=2)
                nc.sync.dma_start(
                    out=a32c,
                    in_=a_d[:].rearrange("(t p) n -> p t n", p=128)[:, cc, :])
                nc.vector.tensor_scalar(out=mn[:, cc, :], in0=a32c, scalar1=-1.0,
                                        scalar2=1.0, op0=OP.mult, op1=OP.add)
            wv1h, wv1l = _load_weight2d(nc, sb, scratch, "Wv1", w_d["Wv1"], F, H * U1, True)
            wo1h, wo1l = _load_weight_kt(nc, sb, scratch, "Wo1", w_d["Wo1"], H * U1 // 128, U1, True)
            Mt = sb.tile([128, NT, N], BF16, tag="Mt", bufs=1)
            for cc in range(NT):
                nc.sync.dma_start(out=Mt[:, :, cc * 128:(cc + 1) * 128],
                                  in_=mn[:, cc, :], transpose=True)

            # ---- weights ----
            wq2, _ = _load_weight2d(nc, sb, scratch, "Wq2", w_d["Wq2"], U1, H * U2, False)
            wk2, _ = _load_weight2d(nc, sb, scratch, "Wk2", w_d["Wk2"], U1, H * U2, False)
            wv2h, wv2l = _load_weight2d(nc, sb, scratch, "Wv2", w_d["Wv2"], U1, H * U2, True)
            wo2h, wo2l = _load_weight_kt(nc, sb, scratch, "Wo2", w_d["Wo2"], H * U2 // 128, U2, True)

            w1 = sb.tile([F, 32], F32, tag="W1", bufs=1)
            nc.sync.dma_start(out=w1, in_=w_d["W1"][:])
            w2 = sb.tile([32, 16], F32, tag="W2", bufs=1)
            nc.sync.dma_start(out=w2, in_=w_d["W2"][:])
            w3 = sb.tile([16, 1], F32, tag="W3", bufs=1)
            nc.sync.dma_start(out=w3, in_=w_d["W3"][:])
            b1 = sb.tile([32, 1], F32, tag="b1", bufs=1)
            nc.sync.dma_start(out=b1, in_=w_d["b1"][:].rearrange("(p x) -> p x", x=1))
            b2 = sb.tile([16, 1], F32, tag="b2", bufs=1)
            nc.sync.dma_start(out=b2, in_=w_d["b2"][:].rearrange("(p x) -> p x", x=1))
            b3 = sb.tile([1, 1], F32, tag="b3", bufs=1)
            nc.sync.dma_start(out=b3, in_=w_d["b3"][:].rearrange("(p x) -> p x", x=1))

            # ---- layers ----
            H1T = sb.tile([U1, N], BF16, tag="H1T", bufs=1)
            _gat_layer(nc, sb, ps_big, ps_small, XT, U1, U1,
                       wq1, wk1, wv1h, wv1l, wo1h, wo1l, Mt, H1T)
            H2T = sb.tile([U2, N], BF16, tag="H2T", bufs=1)
            _gat_layer(nc, sb, ps_big, ps_small, H1T, U2, U2,
                       wq2, wk2, wv2h, wv2l, wo2h, wo2l, Mt, H2T)

            # ---- mean pool + MLP ----
            hs2 = sb.tile([U2, 2], F32, tag="hsum2", bufs=1)
            for q in range(2):
                nc.vector.reduce_sum(out=hs2[:, q:q + 1],
                                     in_=H2T[:, q * 512:(q + 1) * 512],
                                     axis=mybir.AxisListType.X)
            hs = sb.tile([U2, 1], F32, tag="hsum", bufs=1)
            nc.vector.tensor_add(out=hs, in0=hs2[:, 0:1], in1=hs2[:, 1:2])
            p1 = ps_small.tile([32, 1], F32, tag="ps_c", bufs=2)
            nc.tensor.matmul(p1, lhsT=w1, rhs=hs, start=True, stop=True)
            a1 = sb.tile([32, 1], F32, tag="a1", bufs=1)
            nc.scalar.activation(out=a1, in_=p1, func=AF.Relu, bias=b1,
                                 scale=1.0 / float(N))
            p2 = ps_small.tile([16, 1], F32, tag="ps_c", bufs=2)
            nc.tensor.matmul(p2, lhsT=w2, rhs=a1, start=True, stop=True)
            a2 = sb.tile([16, 1], F32, tag="a2", bufs=1)
            nc.scalar.activation(out=a2, in_=p2, func=AF.Relu, bias=b2)
            p3 = ps_small.tile([1, 1], F32, tag="ps_c", bufs=2)
            nc.tensor.matmul(p3, lhsT=w3, rhs=a2, start=True, stop=True)
            yt = sb.tile([1, 1], F32, tag="yt", bufs=1)
            nc.vector.tensor_add(out=yt, in0=p3, in1=b3)
            nc.sync.dma_start(out=y_d[:], in_=yt)

    nc.compile()
    return nc


_NC = None


def _get_nc():
    global _NC
    if _NC is None:
        _NC = build_nc()
    return _NC


def make_in_maps(inputs):
    in_maps = []
    for i in range(B):
        m = {"X": np.ascontiguousarray(np.asarray(inputs["X"][i], dtype=np.float32)),
             "A": np.ascontiguousarray(np.asarray(inputs["A"][i], dtype=np.float32))}
        for k in WEIGHT_NAMES:
            m[k] = np.ascontiguousarray(np.asarray(inputs[k], dtype=np.float32))
        in_maps.append(m)
    return in_maps


def run(inputs, trace=False):
    nc = _get_nc()
    res = run_bass_kernel_spmd(nc, make_in_maps(inputs), list(range(B)), trace=trace)
    y = np.stack([res.results[i]["y"][0] for i in range(B)], axis=0)
    return y.astype(np.float32), res


def kernel(**inputs):
    y, _ = run(inputs, trace=False)
    return y

